# revision 4
# baseline (speedup 1.0000x reference)
"""Grouped triplet loss on 8 trn2 NeuronCores.

Strategy (data-parallel over A rows, hint-compliant):
  - Each core takes a 1024-row block of A, full B (column-rotated so the
    diagonal of the distance matrix lands at core-independent positions).
  - L2 normalization of A-block and B on device.
  - One fused matmul per (row-tile, col-chunk) computes the *masked* squared
    distance directly in PSUM via extended feature vectors:
        F_A = [ a_i (32) | 1 | -BIG*onehot(label_i) (32) ]   (K = 65)
        F_B = [ -2*b_j   | 2+BIG |      onehot(label_j)  ]
    so PSUM = 2 - 2*a.b + BIG*(1 - same_group).
  - A tiny bf16 identity matmul accumulates +BIG on the diagonal (self-pair
    exclusion).
  - DVE min-reduces PSUM (4 banks per op); rows with min >= TH had no valid
    negative -> dist_neg = 0 (matches torch "skip groups of size < 2").
  - losses = relu(dist_pos - dist_neg + margin); host averages.

Host-side work is limited to sharding/layout: slicing, row-rotation, (t p)
tiling, and one-hot encoding of the integer labels. All float math happens
on device.
"""

import numpy as np

import concourse.bass as bass
import concourse.mybir as mybir
from concourse.tile import TileContext
from concourse.bass_utils import run_bass_kernel_spmd

N, D, G = 8192, 32, 32
NCORES = 8
RPC = N // NCORES      # rows per core = 1024
RT = RPC // 128        # row tiles per core = 8
CT = N // 128          # column tiles = 64
NCHUNK = N // 512      # matmul column chunks = 16
BIG = 64.0
TH = 32.0
MARGIN = 1.0

F32 = mybir.dt.float32
BF16 = mybir.dt.bfloat16
AF = mybir.ActivationFunctionType
ALU = mybir.AluOpType
AX = mybir.AxisListType

MM_DT = F32  # matmul feature dtype (float32 | float32r)

_MAX_DRAIN_WAITS = 1


def _split_drain_waits(nc):
    """This container's walrus rejects any instruction with >1 sem-wait.
    Hoist excess waits onto preceding same-engine single-wait Drains."""
    nsplit = 0
    for f in nc.m.functions:
        for bb in f.blocks:
            new_insts = []
            for inst in bb.instructions:
                si = inst.sync_info
                waits = list(si.on_wait) if si and si.on_wait else []
                if len(waits) > _MAX_DRAIN_WAITS:
                    extra, keep = waits[:-_MAX_DRAIN_WAITS], waits[-_MAX_DRAIN_WAITS:]
                    for w in extra:
                        d = mybir.InstDrain(
                            name=f"{inst.name}-swsplit{nsplit}",
                            engine=inst.engine,
                            ins=[],
                            outs=[],
                            sync_info=mybir.SyncInfo(on_wait=[w], on_update=[]),
                        )
                        nsplit += 1
                        nc.register_instruction(d, overwrite=True)
                        new_insts.append(d)
                    si.on_wait = keep
                new_insts.append(inst)
            bb.instructions[:] = new_insts


def _build_nc():
    import ml_dtypes

    nc = bass.Bass()

    a_in = nc.dram_tensor("a", [128, RT * D], F32, kind="ExternalInput")
    b_in = nc.dram_tensor("b", [128, CT * D], F32, kind="ExternalInput")
    oha_in = nc.dram_tensor("oha", [G, RPC], F32, kind="ExternalInput")
    ohb_in = nc.dram_tensor("ohb", [G, N], F32, kind="ExternalInput")
    out = nc.dram_tensor("losses", [128, RT], F32, kind="ExternalOutput")

    ident_np = np.eye(128, dtype=np.float32)
    sel_np = np.zeros((128, 1024), dtype=np.float32)
    sel_np[np.arange(128), 512 + np.arange(128)] = 1.0
    bigi_np = (BIG * np.eye(128)).astype(ml_dtypes.bfloat16)
    ident_d = nc.inline_tensor(ident_np, name="identc")
    sel_d = nc.inline_tensor(sel_np.astype(ml_dtypes.bfloat16), name="selc")
    bigi_d = nc.inline_tensor(bigi_np, name="bigic")

    with TileContext(nc) as tc:
        with (
            tc.tile_pool(name="const", bufs=1) as cpool,
            tc.tile_pool(name="work", bufs=1) as wpool,
            tc.tile_pool(name="ps", bufs=2, space="PSUM") as pspool,
        ):
            # ---- constants -------------------------------------------------
            ident = cpool.tile([128, 128], F32, tag="ident")
            nc.sync.dma_start(out=ident[:], in_=ident_d[:, :])
            sel = cpool.tile([128, 1024], BF16, tag="sel")
            nc.sync.dma_start(out=sel[:], in_=sel_d[:, :])
            bigi = cpool.tile([128, 128], BF16, tag="bigi")
            nc.sync.dma_start(out=bigi[:], in_=bigi_d[:, :])

            # ---- raw loads -------------------------------------------------
            tA = wpool.tile([128, RT * D], F32, tag="tA")
            nc.sync.dma_start(out=tA[:], in_=a_in[:, :])
            tB = wpool.tile([128, CT * D], F32, tag="tB")
            # split into 2 DMAs to use more queues
            nc.sync.dma_start(out=tB[:, : CT * D // 2], in_=b_in[:, : CT * D // 2])
            nc.sync.dma_start(out=tB[:, CT * D // 2 :], in_=b_in[:, CT * D // 2 :])

            fA = cpool.tile([G + 33, RPC], F32, tag="fA")
            fB = cpool.tile([G + 33, N], F32, tag="fB")
            nc.sync.dma_start(out=fA[33:65, :], in_=oha_in[:, :])
            nc.sync.dma_start(out=fB[33:65, : N // 2], in_=ohb_in[:, : N // 2])
            nc.sync.dma_start(out=fB[33:65, N // 2 :], in_=ohb_in[:, N // 2 :])
            nc.gpsimd.memset(fA[32:33, :], 1.0)
            nc.gpsimd.memset(fB[32:33, :], 2.0 + BIG)

            # ---- normalize A block ----------------------------------------
            tA3 = tA[:, :].rearrange("p (t d) -> p t d", d=D)
            sqA = wpool.tile([128, RT * D], F32, tag="sqA")
            nc.scalar.activation(sqA[:], tA[:], AF.Square)
            ssA = wpool.tile([128, RT], F32, tag="ssA")
            nc.vector.tensor_reduce(
                ssA[:], sqA[:, :].rearrange("p (t d) -> p t d", d=D), axis=AX.X, op=ALU.add
            )
            nA = wpool.tile([128, RT], F32, tag="nA")
            nc.scalar.activation(nA[:], ssA[:], AF.Sqrt)
            rA = wpool.tile([128, RT], F32, tag="rA")
            nc.vector.reciprocal(rA[:], nA[:])
            an = wpool.tile([128, RT * D], F32, tag="an")
            an3 = an[:, :].rearrange("p (t d) -> p t d", d=D)
            nc.vector.tensor_tensor(
                an3, tA3, rA[:, :].broadcast_to([128, RT, D]), op=ALU.mult
            )

            # ---- normalize B (scaled by -2 for features) -------------------
            tB3 = tB[:, :].rearrange("p (t d) -> p t d", d=D)
            sqB = wpool.tile([128, CT * D], F32, tag="sqB")
            nc.scalar.activation(sqB[:], tB[:], AF.Square)
            ssB = wpool.tile([128, CT], F32, tag="ssB")
            nc.vector.tensor_reduce(
                ssB[:], sqB[:, :].rearrange("p (t d) -> p t d", d=D), axis=AX.X, op=ALU.add
            )
            nB = wpool.tile([128, CT], F32, tag="nB")
            nc.scalar.activation(nB[:], ssB[:], AF.Sqrt)
            rB = wpool.tile([128, CT], F32, tag="rB")
            nc.vector.reciprocal(rB[:], nB[:])
            rBm2 = wpool.tile([128, CT], F32, tag="rBm2")
            nc.vector.tensor_scalar(rBm2[:], rB[:], -2.0, None, op0=ALU.mult)
            bn2 = wpool.tile([128, CT * D], F32, tag="bn2")
            bn23 = bn2[:, :].rearrange("p (t d) -> p t d", d=D)
            nc.vector.tensor_tensor(
                bn23, tB3, rBm2[:, :].broadcast_to([128, CT, D]), op=ALU.mult
            )

            # ---- transpose an -> fA[0:32, :] ------------------------------
            psA = pspool.tile([32, RPC], F32, tag="ps")
            for r in range(RT):
                nc.tensor.transpose(psA[:, r * 128 : (r + 1) * 128], an3[:, r, :], ident[:])
            nc.scalar.copy(fA[0:32, :], psA[:, :])

            # ---- transpose bn2 -> fB[0:32, :] ------------------------------
            for grp in range(CT // 16):
                psB = pspool.tile([32, 16 * 128], F32, tag="ps")
                for k in range(16):
                    t = grp * 16 + k
                    nc.tensor.transpose(
                        psB[:, k * 128 : (k + 1) * 128], bn23[:, t, :], ident[:]
                    )
                nc.scalar.copy(fB[0:32, grp * 2048 : (grp + 1) * 2048], psB[:, :])

            # ---- dist_pos for own rows (first RT tiles of rotated B) ------
            bno = wpool.tile([128, RT * D], F32, tag="bno")
            bno3 = bno[:, :].rearrange("p (t d) -> p t d", d=D)
            nc.vector.tensor_tensor(
                bno3, tB3[:, 0:RT, :], rB[:, 0:RT].broadcast_to([128, RT, D]), op=ALU.mult
            )
            dd = wpool.tile([128, RT * D], F32, tag="dd")
            nc.vector.tensor_tensor(dd[:], an[:], bno[:], op=ALU.subtract)
            sqd = wpool.tile([128, RT * D], F32, tag="sqd")
            nc.scalar.activation(sqd[:], dd[:], AF.Square)
            dp2 = wpool.tile([128, RT], F32, tag="dp2")
            nc.vector.tensor_reduce(
                dp2[:], sqd[:, :].rearrange("p (t d) -> p t d", d=D), axis=AX.X, op=ALU.add
            )
            dpos = wpool.tile([128, RT], F32, tag="dpos")
            nc.scalar.activation(dpos[:], dp2[:], AF.Sqrt)

            # ---- main loop: fused matmul + masked min ----------------------
            mpart = wpool.tile([128, RT * 4], F32, tag="mpart")
            for r in range(RT):
                lhsT = fA[:, r * 128 : (r + 1) * 128]
                for q in range(4):
                    P4 = pspool.tile([128, 2048], F32, tag="ps")
                    for j in range(4):
                        c = q * 4 + j
                        is_diag = q == 0 and j == r // 4
                        nc.tensor.matmul(
                            P4[:, j * 512 : (j + 1) * 512],
                            lhsT,
                            fB[:, c * 512 : (c + 1) * 512],
                            start=True,
                            stop=not is_diag,
                        )
                        if is_diag:
                            off = (r % 4) * 128
                            nc.tensor.matmul(
                                P4[:, j * 512 : (j + 1) * 512],
                                bigi[:],
                                sel[:, 512 - off : 1024 - off],
                                start=False,
                                stop=True,
                            )
                    nc.vector.tensor_reduce(
                        mpart[:, r * 4 + q : r * 4 + q + 1],
                        P4[:, :].rearrange("p (f c) -> p f c", c=512),
                        axis=AX.XY,
                        op=ALU.min,
                    )

            # ---- finalize --------------------------------------------------
            m = wpool.tile([128, RT], F32, tag="m")
            nc.vector.tensor_reduce(
                m[:], mpart[:, :].rearrange("p (r q) -> p r q", q=4), axis=AX.X, op=ALU.min
            )
            mc = wpool.tile([128, RT], F32, tag="mc")
            nc.vector.tensor_scalar(mc[:], m[:], 0.0, None, op0=ALU.max)
            sn = wpool.tile([128, RT], F32, tag="sn")
            nc.scalar.activation(sn[:], mc[:], AF.Sqrt)
            valid = wpool.tile([128, RT], F32, tag="valid")
            nc.vector.tensor_scalar(valid[:], m[:], TH, None, op0=ALU.is_lt)
            dn = wpool.tile([128, RT], F32, tag="dn")
            nc.vector.tensor_tensor(dn[:], sn[:], valid[:], op=ALU.mult)
            pre = wpool.tile([128, RT], F32, tag="pre")
            nc.vector.tensor_tensor(pre[:], dpos[:], dn[:], op=ALU.subtract)
            losses = wpool.tile([128, RT], F32, tag="losses")
            nc.scalar.activation(losses[:], pre[:], AF.Relu, bias=MARGIN)
            nc.sync.dma_start(out=out[:, :], in_=losses[:])

    _split_drain_waits(nc)
    return nc


_NC_CACHE = None


def _get_nc():
    global _NC_CACHE
    if _NC_CACHE is None:
        _NC_CACHE = _build_nc()
    return _NC_CACHE


def _tile_tp(x):
    """[R, 32] rows -> [128, (R/128)*32] with row t*128+p on partition p."""
    r = x.shape[0]
    return (
        np.ascontiguousarray(
            x.reshape(r // 128, 128, D).transpose(1, 0, 2).reshape(128, (r // 128) * D)
        )
    )


def kernel(A=None, B=None, labels=None, **_unused):
    A = np.asarray(A, dtype=np.float32)
    B = np.asarray(B, dtype=np.float32)
    lab = np.asarray(labels).astype(np.int32)

    eye = np.arange(G, dtype=np.int32)
    in_maps = []
    for c in range(NCORES):
        rows = slice(c * RPC, (c + 1) * RPC)
        a_c = _tile_tp(A[rows])
        b_rot = np.roll(B, -c * RPC, axis=0)
        lab_rot = np.roll(lab, -c * RPC)
        b_c = _tile_tp(b_rot)
        oha = (-BIG) * (lab[rows][None, :] == eye[:, None]).astype(np.float32)
        ohb = (lab_rot[None, :] == eye[:, None]).astype(np.float32)
        in_maps.append(
            {
                "a": a_c,
                "b": b_c,
                "oha": np.ascontiguousarray(oha),
                "ohb": np.ascontiguousarray(ohb),
            }
        )

    global _last_in_maps
    _last_in_maps = in_maps
    nc = _get_nc()
    res = run_bass_kernel_spmd(nc, in_maps, list(range(NCORES)))
    total = 0.0
    for c in range(NCORES):
        lo = res.results[c]["losses"]  # [128, RT]; [p, r] = loss of row r*128+p
        total += float(lo.sum(dtype=np.float64))
    return np.float32(total / N)


# revision 12
# speedup vs baseline: 1.4334x; 1.4334x over previous
"""Grouped triplet loss on 8 trn2 NeuronCores.

Strategy (data-parallel over A rows, hint-compliant):
  - Each core takes a 1024-row block of A, full B (column-rotated so the
    diagonal of the distance matrix lands at core-independent positions).
  - L2 normalization of A-block and B on device.
  - One fused matmul per (row-tile, col-chunk) computes the *masked* squared
    distance directly in PSUM via extended feature vectors:
        F_A = [ a_i (32) | 1 | -BIG*onehot(label_i) (32) ]   (K = 65)
        F_B = [ -2*b_j   | 2+BIG |      onehot(label_j)  ]
    so PSUM = 2 - 2*a.b + BIG*(1 - same_group).
  - A tiny bf16 identity matmul accumulates +BIG on the diagonal (self-pair
    exclusion).
  - DVE min-reduces PSUM (4 banks per op); rows with min >= TH had no valid
    negative -> dist_neg = 0 (matches torch "skip groups of size < 2").
  - losses = relu(dist_pos - dist_neg + margin); host averages.

Host-side work is limited to sharding/layout: slicing, row-rotation, (t p)
tiling, and one-hot encoding of the integer labels. All float math happens
on device.
"""

import numpy as np

import concourse.bass as bass
import concourse.mybir as mybir
from concourse.tile import TileContext
from concourse.bass_utils import run_bass_kernel_spmd

N, D, G = 8192, 32, 32
NCORES = 8
RPC = N // NCORES      # rows per core = 1024
RT = RPC // 128        # row tiles per core = 8
CT = N // 128          # column tiles = 64
NCHUNK = N // 512      # matmul column chunks = 16
BIG = 64.0
TH = 32.0
MARGIN = 1.0

F32 = mybir.dt.float32
BF16 = mybir.dt.bfloat16
AF = mybir.ActivationFunctionType
ALU = mybir.AluOpType
AX = mybir.AxisListType

MM_DT = mybir.dt.float32r  # matmul feature dtype (float32 | float32r)

_MAX_DRAIN_WAITS = 1


def _split_drain_waits(nc):
    """This container's walrus rejects any instruction with >1 sem-wait.
    Hoist excess waits onto preceding same-engine single-wait Drains."""
    nsplit = 0
    for f in nc.m.functions:
        for bb in f.blocks:
            new_insts = []
            for inst in bb.instructions:
                si = inst.sync_info
                waits = list(si.on_wait) if si and si.on_wait else []
                if len(waits) > _MAX_DRAIN_WAITS:
                    extra, keep = waits[:-_MAX_DRAIN_WAITS], waits[-_MAX_DRAIN_WAITS:]
                    for w in extra:
                        d = mybir.InstDrain(
                            name=f"{inst.name}-swsplit{nsplit}",
                            engine=inst.engine,
                            ins=[],
                            outs=[],
                            sync_info=mybir.SyncInfo(on_wait=[w], on_update=[]),
                        )
                        nsplit += 1
                        nc.register_instruction(d, overwrite=True)
                        new_insts.append(d)
                    si.on_wait = keep
                new_insts.append(inst)
            bb.instructions[:] = new_insts


def _build_nc():
    import ml_dtypes

    nc = bass.Bass()

    a_in = nc.dram_tensor("a", [128, RT * D], F32, kind="ExternalInput")
    b_in = nc.dram_tensor("b", [128, CT * D], F32, kind="ExternalInput")
    # row 0: constant feature (1 for A, 2+BIG for B); rows 1..32: one-hot
    oha_in = nc.dram_tensor("oha", [G + 1, RPC], MM_DT, kind="ExternalInput")
    ohb_in = nc.dram_tensor("ohb", [G + 1, N], MM_DT, kind="ExternalInput")
    out = nc.dram_tensor("losses", [128, RT], F32, kind="ExternalOutput")

    ident_np = np.eye(128, dtype=np.float32)
    sel_np = np.zeros((128, 1024), dtype=np.float32)
    sel_np[np.arange(128), 512 + np.arange(128)] = 1.0
    bigi_np = (BIG * np.eye(128)).astype(ml_dtypes.bfloat16)
    ident_d = nc.inline_tensor(ident_np, name="identc")
    sel_d = nc.inline_tensor(sel_np.astype(ml_dtypes.bfloat16), name="selc")
    bigi_d = nc.inline_tensor(bigi_np, name="bigic")

    with TileContext(nc) as tc:
        with (
            tc.tile_pool(name="const", bufs=1) as cpool,
            tc.tile_pool(name="work", bufs=1) as wpool,
            tc.tile_pool(name="ps", bufs=2, space="PSUM") as pspool,
        ):
            # ---- constants -------------------------------------------------
            ident = cpool.tile([128, 128], F32, tag="ident")
            nc.sync.dma_start(out=ident[:], in_=ident_d[:, :])
            sel = cpool.tile([128, 1024], BF16, tag="sel")
            nc.sync.dma_start(out=sel[:], in_=sel_d[:, :])
            bigi = cpool.tile([128, 128], BF16, tag="bigi")
            nc.sync.dma_start(out=bigi[:], in_=bigi_d[:, :])

            # ---- raw loads -------------------------------------------------
            tA = wpool.tile([128, RT * D], F32, tag="tA")
            nc.sync.dma_start(out=tA[:], in_=a_in[:, :])
            tB = wpool.tile([128, CT * D], F32, tag="tB")
            # split into 2 DMAs to use more queues
            nc.sync.dma_start(out=tB[:, : CT * D // 2], in_=b_in[:, : CT * D // 2])
            nc.sync.dma_start(out=tB[:, CT * D // 2 :], in_=b_in[:, CT * D // 2 :])

            fA = cpool.tile([G + 33, RPC], MM_DT, tag="fA")
            fB = cpool.tile([G + 33, N], MM_DT, tag="fB")
            nc.sync.dma_start(out=fA[32:65, :], in_=oha_in[:, :])
            nc.sync.dma_start(out=fB[32:65, : N // 2], in_=ohb_in[:, : N // 2])
            nc.sync.dma_start(out=fB[32:65, N // 2 :], in_=ohb_in[:, N // 2 :])

            # ---- normalize A block ----------------------------------------
            tA3 = tA[:, :].rearrange("p (t d) -> p t d", d=D)
            sqA = wpool.tile([128, RT * D], F32, tag="sqA")
            nc.scalar.activation(sqA[:], tA[:], AF.Square)
            ssA = wpool.tile([128, RT], F32, tag="ssA")
            nc.vector.tensor_reduce(
                ssA[:], sqA[:, :].rearrange("p (t d) -> p t d", d=D), axis=AX.X, op=ALU.add
            )
            nA = wpool.tile([128, RT], F32, tag="nA")
            nc.scalar.activation(nA[:], ssA[:], AF.Sqrt)
            rA = wpool.tile([128, RT], F32, tag="rA")
            nc.vector.reciprocal(rA[:], nA[:])
            an = wpool.tile([128, RT * D], F32, tag="an")
            an3 = an[:, :].rearrange("p (t d) -> p t d", d=D)
            nc.vector.tensor_tensor(
                an3, tA3, rA[:, :].broadcast_to([128, RT, D]), op=ALU.mult
            )

            # ---- normalize B (scaled by -2 for features) -------------------
            tB3 = tB[:, :].rearrange("p (t d) -> p t d", d=D)
            sqB = wpool.tile([128, CT * D], F32, tag="sqB")
            nc.scalar.activation(sqB[:], tB[:], AF.Square)
            ssB = wpool.tile([128, CT], F32, tag="ssB")
            nc.vector.tensor_reduce(
                ssB[:], sqB[:, :].rearrange("p (t d) -> p t d", d=D), axis=AX.X, op=ALU.add
            )
            nB = wpool.tile([128, CT], F32, tag="nB")
            nc.scalar.activation(nB[:], ssB[:], AF.Sqrt)
            rB = wpool.tile([128, CT], F32, tag="rB")
            nc.vector.reciprocal(rB[:], nB[:])
            rBm2 = wpool.tile([128, CT], F32, tag="rBm2")
            nc.vector.tensor_scalar(rBm2[:], rB[:], -2.0, None, op0=ALU.mult)
            bn2 = wpool.tile([128, CT * D], F32, tag="bn2")
            bn23 = bn2[:, :].rearrange("p (t d) -> p t d", d=D)
            nc.vector.tensor_tensor(
                bn23, tB3, rBm2[:, :].broadcast_to([128, CT, D]), op=ALU.mult
            )

            # ---- transpose an -> fA[0:32, :] ------------------------------
            psA = pspool.tile([32, RPC], F32, tag="ps")
            for r in range(RT):
                nc.tensor.transpose(psA[:, r * 128 : (r + 1) * 128], an3[:, r, :], ident[:])
            nc.scalar.copy(fA[0:32, :], psA[:, :])

            # ---- transpose bn2 -> fB[0:32, :] ------------------------------
            for grp in range(CT // 16):
                psB = pspool.tile([32, 16 * 128], F32, tag="ps")
                for k in range(16):
                    t = grp * 16 + k
                    nc.tensor.transpose(
                        psB[:, k * 128 : (k + 1) * 128], bn23[:, t, :], ident[:]
                    )
                nc.scalar.copy(fB[0:32, grp * 2048 : (grp + 1) * 2048], psB[:, :])

            # ---- dist_pos for own rows (first RT tiles of rotated B) ------
            bno = wpool.tile([128, RT * D], F32, tag="bno")
            bno3 = bno[:, :].rearrange("p (t d) -> p t d", d=D)
            nc.vector.tensor_tensor(
                bno3, tB3[:, 0:RT, :], rB[:, 0:RT].broadcast_to([128, RT, D]), op=ALU.mult
            )
            dd = wpool.tile([128, RT * D], F32, tag="dd")
            nc.vector.tensor_tensor(dd[:], an[:], bno[:], op=ALU.subtract)
            sqd = wpool.tile([128, RT * D], F32, tag="sqd")
            nc.scalar.activation(sqd[:], dd[:], AF.Square)
            dp2 = wpool.tile([128, RT], F32, tag="dp2")
            nc.vector.tensor_reduce(
                dp2[:], sqd[:, :].rearrange("p (t d) -> p t d", d=D), axis=AX.X, op=ALU.add
            )
            dpos = wpool.tile([128, RT], F32, tag="dpos")
            nc.scalar.activation(dpos[:], dp2[:], AF.Sqrt)

            # ---- main loop: fused matmul + masked min ----------------------
            mpart = wpool.tile([128, RT * 4], F32, tag="mpart")
            for r in range(RT):
                lhsT = fA[:, r * 128 : (r + 1) * 128]
                for q in range(4):
                    P4 = pspool.tile([128, 2048], F32, tag="ps")
                    for j in range(4):
                        c = q * 4 + j
                        is_diag = q == 0 and j == r // 4
                        nc.tensor.matmul(
                            P4[:, j * 512 : (j + 1) * 512],
                            lhsT,
                            fB[:, c * 512 : (c + 1) * 512],
                            start=True,
                            stop=not is_diag,
                        )
                        if is_diag:
                            off = (r % 4) * 128
                            nc.tensor.matmul(
                                P4[:, j * 512 : (j + 1) * 512],
                                bigi[:],
                                sel[:, 512 - off : 1024 - off],
                                start=False,
                                stop=True,
                            )
                    nc.vector.tensor_reduce(
                        mpart[:, r * 4 + q : r * 4 + q + 1],
                        P4[:, :].rearrange("p (f c) -> p f c", c=512),
                        axis=AX.XY,
                        op=ALU.min,
                    )

            # ---- finalize --------------------------------------------------
            m = wpool.tile([128, RT], F32, tag="m")
            nc.vector.tensor_reduce(
                m[:], mpart[:, :].rearrange("p (r q) -> p r q", q=4), axis=AX.X, op=ALU.min
            )
            mc = wpool.tile([128, RT], F32, tag="mc")
            nc.vector.tensor_scalar(mc[:], m[:], 0.0, None, op0=ALU.max)
            sn = wpool.tile([128, RT], F32, tag="sn")
            nc.scalar.activation(sn[:], mc[:], AF.Sqrt)
            valid = wpool.tile([128, RT], F32, tag="valid")
            nc.vector.tensor_scalar(valid[:], m[:], TH, None, op0=ALU.is_lt)
            dn = wpool.tile([128, RT], F32, tag="dn")
            nc.vector.tensor_tensor(dn[:], sn[:], valid[:], op=ALU.mult)
            pre = wpool.tile([128, RT], F32, tag="pre")
            nc.vector.tensor_tensor(pre[:], dpos[:], dn[:], op=ALU.subtract)
            losses = wpool.tile([128, RT], F32, tag="losses")
            nc.scalar.activation(losses[:], pre[:], AF.Relu, bias=MARGIN)
            nc.sync.dma_start(out=out[:, :], in_=losses[:])

    _split_drain_waits(nc)
    return nc


_NC_CACHE = None


def _get_nc():
    global _NC_CACHE
    if _NC_CACHE is None:
        _NC_CACHE = _build_nc()
    return _NC_CACHE


def _tile_tp(x):
    """[R, 32] rows -> [128, (R/128)*32] with row t*128+p on partition p."""
    r = x.shape[0]
    return (
        np.ascontiguousarray(
            x.reshape(r // 128, 128, D).transpose(1, 0, 2).reshape(128, (r // 128) * D)
        )
    )


def kernel(A=None, B=None, labels=None, **_unused):
    A = np.asarray(A, dtype=np.float32)
    B = np.asarray(B, dtype=np.float32)
    lab = np.asarray(labels).astype(np.int32)

    eye = np.arange(G, dtype=np.int32)
    in_maps = []
    for c in range(NCORES):
        rows = slice(c * RPC, (c + 1) * RPC)
        a_c = _tile_tp(A[rows])
        b_rot = np.roll(B, -c * RPC, axis=0)
        lab_rot = np.roll(lab, -c * RPC)
        b_c = _tile_tp(b_rot)
        oha = np.concatenate(
            [
                np.ones((1, RPC), np.float32),
                (-BIG) * (lab[rows][None, :] == eye[:, None]).astype(np.float32),
            ]
        )
        ohb = np.concatenate(
            [
                np.full((1, N), 2.0 + BIG, np.float32),
                (lab_rot[None, :] == eye[:, None]).astype(np.float32),
            ]
        )
        in_maps.append(
            {
                "a": a_c,
                "b": b_c,
                "oha": np.ascontiguousarray(oha),
                "ohb": np.ascontiguousarray(ohb),
            }
        )

    global _last_in_maps
    _last_in_maps = in_maps
    nc = _get_nc()
    res = run_bass_kernel_spmd(nc, in_maps, list(range(NCORES)))
    total = 0.0
    for c in range(NCORES):
        lo = res.results[c]["losses"]  # [128, RT]; [p, r] = loss of row r*128+p
        total += float(lo.sum(dtype=np.float64))
    return np.float32(total / N)


# revision 15
# speedup vs baseline: 4.2385x; 2.9569x over previous
"""Grouped triplet loss on 8 trn2 NeuronCores.

Strategy (data-parallel over A rows, hint-compliant):
  - Each core takes a 1024-row block of A, full B (column-rotated so the
    diagonal of the distance matrix lands at core-independent positions).
  - L2 normalization of A-block and B on device.
  - One fused matmul per (row-tile, col-chunk) computes the *masked* squared
    distance directly in PSUM via extended feature vectors:
        F_A = [ a_i (32) | 1 | -BIG*onehot(label_i) (32) ]   (K = 65)
        F_B = [ -2*b_j   | 2+BIG |      onehot(label_j)  ]
    so PSUM = 2 - 2*a.b + BIG*(1 - same_group).
  - A tiny bf16 identity matmul accumulates +BIG on the diagonal (self-pair
    exclusion).
  - DVE min-reduces PSUM (4 banks per op); rows with min >= TH had no valid
    negative -> dist_neg = 0 (matches torch "skip groups of size < 2").
  - losses = relu(dist_pos - dist_neg + margin); host averages.

Host-side work is limited to sharding/layout: slicing, row-rotation, (t p)
tiling, and one-hot encoding of the integer labels. All float math happens
on device.
"""

import numpy as np

import concourse.bass as bass
import concourse.mybir as mybir
from concourse.tile import TileContext
from concourse.bass_utils import run_bass_kernel_spmd

N, D, G = 8192, 32, 32
NCORES = 8
RPC = N // NCORES      # rows per core = 1024
RT = RPC // 128        # row tiles per core = 8
CT = N // 128          # column tiles = 64
NCHUNK = N // 512      # matmul column chunks = 16
BIG = 64.0
TH = 32.0
MARGIN = 1.0

F32 = mybir.dt.float32
BF16 = mybir.dt.bfloat16
AF = mybir.ActivationFunctionType
ALU = mybir.AluOpType
AX = mybir.AxisListType

MM_DT = mybir.dt.float32r  # matmul feature dtype (float32 | float32r)

_MAX_DRAIN_WAITS = 1


def _split_drain_waits(nc):
    """This container's walrus rejects any instruction with >1 sem-wait.
    Hoist excess waits onto preceding same-engine single-wait Drains."""
    nsplit = 0
    for f in nc.m.functions:
        for bb in f.blocks:
            new_insts = []
            for inst in bb.instructions:
                si = inst.sync_info
                waits = list(si.on_wait) if si and si.on_wait else []
                if len(waits) > _MAX_DRAIN_WAITS:
                    extra, keep = waits[:-_MAX_DRAIN_WAITS], waits[-_MAX_DRAIN_WAITS:]
                    for w in extra:
                        d = mybir.InstDrain(
                            name=f"{inst.name}-swsplit{nsplit}",
                            engine=inst.engine,
                            ins=[],
                            outs=[],
                            sync_info=mybir.SyncInfo(on_wait=[w], on_update=[]),
                        )
                        nsplit += 1
                        nc.register_instruction(d, overwrite=True)
                        new_insts.append(d)
                    si.on_wait = keep
                new_insts.append(inst)
            bb.instructions[:] = new_insts


def _build_nc():
    import ml_dtypes

    nc = bass.Bass()

    a_in = nc.dram_tensor("a", [128, RT * D], F32, kind="ExternalInput")
    b_in = nc.dram_tensor("b", [128, CT * D], F32, kind="ExternalInput")
    # row 0: constant feature (1 for A, 2+BIG for B); rows 1..32: one-hot
    oha_in = nc.dram_tensor("oha", [G + 1, RPC], MM_DT, kind="ExternalInput")
    ohb_in = nc.dram_tensor("ohb", [G + 1, N], MM_DT, kind="ExternalInput")
    out = nc.dram_tensor("losses", [128, RT], F32, kind="ExternalOutput")

    ident_np = np.eye(128, dtype=np.float32)
    sel_np = np.zeros((128, 1024), dtype=np.float32)
    sel_np[np.arange(128), 512 + np.arange(128)] = 1.0
    bigi_np = (BIG * np.eye(128)).astype(ml_dtypes.bfloat16)
    ident_d = nc.inline_tensor(ident_np, name="identc")
    sel_d = nc.inline_tensor(sel_np.astype(ml_dtypes.bfloat16), name="selc")
    bigi_d = nc.inline_tensor(bigi_np, name="bigic")

    with TileContext(nc) as tc:
        with (
            tc.tile_pool(name="const", bufs=1) as cpool,
            tc.tile_pool(name="work", bufs=1) as wpool,
            tc.tile_pool(name="ps", bufs=2, space="PSUM") as pspool,
        ):
            # ---- constants -------------------------------------------------
            ident = cpool.tile([128, 128], F32, tag="ident")
            nc.sync.dma_start(out=ident[:], in_=ident_d[:, :])
            sel = cpool.tile([128, 1024], BF16, tag="sel")
            nc.sync.dma_start(out=sel[:], in_=sel_d[:, :])
            bigi = cpool.tile([128, 128], BF16, tag="bigi")
            nc.sync.dma_start(out=bigi[:], in_=bigi_d[:, :])

            # ---- raw loads -------------------------------------------------
            tA = wpool.tile([128, RT * D], F32, tag="tA")
            nc.sync.dma_start(out=tA[:], in_=a_in[:, :])
            tB = wpool.tile([128, CT * D], F32, tag="tB")
            # split into 2 DMAs to use more queues
            nc.sync.dma_start(out=tB[:, : CT * D // 2], in_=b_in[:, : CT * D // 2])
            nc.sync.dma_start(out=tB[:, CT * D // 2 :], in_=b_in[:, CT * D // 2 :])

            fA = cpool.tile([G + 33, RPC], MM_DT, tag="fA")
            fB = cpool.tile([G + 33, N], MM_DT, tag="fB")
            nc.sync.dma_start(out=fA[32:65, :], in_=oha_in[:, :])
            nc.sync.dma_start(out=fB[32:65, : N // 2], in_=ohb_in[:, : N // 2])
            nc.sync.dma_start(out=fB[32:65, N // 2 :], in_=ohb_in[:, N // 2 :])

            # ---- normalize A block ----------------------------------------
            tA3 = tA[:, :].rearrange("p (t d) -> p t d", d=D)
            sqA = wpool.tile([128, RT * D], F32, tag="sqA")
            nc.scalar.activation(sqA[:], tA[:], AF.Square)
            ssA = wpool.tile([128, RT], F32, tag="ssA")
            nc.vector.tensor_reduce(
                ssA[:], sqA[:, :].rearrange("p (t d) -> p t d", d=D), axis=AX.X, op=ALU.add
            )
            nA = wpool.tile([128, RT], F32, tag="nA")
            nc.scalar.activation(nA[:], ssA[:], AF.Sqrt)
            rA = wpool.tile([128, RT], F32, tag="rA")
            nc.vector.reciprocal(rA[:], nA[:])
            an = wpool.tile([128, RT * D], F32, tag="an")
            an3 = an[:, :].rearrange("p (t d) -> p t d", d=D)
            nc.vector.tensor_tensor(
                an3, tA3, rA[:, :].broadcast_to([128, RT, D]), op=ALU.mult
            )

            # ---- normalize B (scaled by -2 for features) -------------------
            tB3 = tB[:, :].rearrange("p (t d) -> p t d", d=D)
            sqB = wpool.tile([128, CT * D], F32, tag="sqB")
            nc.scalar.activation(sqB[:], tB[:], AF.Square)
            ssB = wpool.tile([128, CT], F32, tag="ssB")
            nc.vector.tensor_reduce(
                ssB[:], sqB[:, :].rearrange("p (t d) -> p t d", d=D), axis=AX.X, op=ALU.add
            )
            nB = wpool.tile([128, CT], F32, tag="nB")
            nc.scalar.activation(nB[:], ssB[:], AF.Sqrt)
            rB = wpool.tile([128, CT], F32, tag="rB")
            nc.vector.reciprocal(rB[:], nB[:])
            rBm2 = wpool.tile([128, CT], F32, tag="rBm2")
            nc.vector.tensor_scalar(rBm2[:], rB[:], -2.0, None, op0=ALU.mult)
            bn2 = wpool.tile([128, CT * D], F32, tag="bn2")
            bn23 = bn2[:, :].rearrange("p (t d) -> p t d", d=D)
            nc.vector.tensor_tensor(
                bn23, tB3, rBm2[:, :].broadcast_to([128, CT, D]), op=ALU.mult
            )

            # ---- transpose an -> fA[0:32, :] ------------------------------
            psA = pspool.tile([32, RPC], F32, tag="ps")
            for r in range(RT):
                nc.tensor.transpose(psA[:, r * 128 : (r + 1) * 128], an3[:, r, :], ident[:])
            nc.scalar.copy(fA[0:32, :], psA[:, :])

            # ---- transpose bn2 -> fB[0:32, :] ------------------------------
            for grp in range(CT // 16):
                psB = pspool.tile([32, 16 * 128], F32, tag="ps")
                for k in range(16):
                    t = grp * 16 + k
                    nc.tensor.transpose(
                        psB[:, k * 128 : (k + 1) * 128], bn23[:, t, :], ident[:]
                    )
                nc.scalar.copy(fB[0:32, grp * 2048 : (grp + 1) * 2048], psB[:, :])

            # ---- dist_pos for own rows (first RT tiles of rotated B) ------
            bno = wpool.tile([128, RT * D], F32, tag="bno")
            bno3 = bno[:, :].rearrange("p (t d) -> p t d", d=D)
            nc.vector.tensor_tensor(
                bno3, tB3[:, 0:RT, :], rB[:, 0:RT].broadcast_to([128, RT, D]), op=ALU.mult
            )
            dd = wpool.tile([128, RT * D], F32, tag="dd")
            nc.vector.tensor_tensor(dd[:], an[:], bno[:], op=ALU.subtract)
            sqd = wpool.tile([128, RT * D], F32, tag="sqd")
            nc.scalar.activation(sqd[:], dd[:], AF.Square)
            dp2 = wpool.tile([128, RT], F32, tag="dp2")
            nc.vector.tensor_reduce(
                dp2[:], sqd[:, :].rearrange("p (t d) -> p t d", d=D), axis=AX.X, op=ALU.add
            )
            dpos = wpool.tile([128, RT], F32, tag="dpos")
            nc.scalar.activation(dpos[:], dp2[:], AF.Sqrt)

            # ---- main loop: fused matmul + masked min ----------------------
            mpart = wpool.tile([128, RT * 4], F32, tag="mpart")
            for r in range(RT):
                lhsT = fA[:, r * 128 : (r + 1) * 128]
                for q in range(4):
                    P4 = pspool.tile([128, 2048], F32, tag="ps")
                    for j in range(4):
                        c = q * 4 + j
                        is_diag = q == 0 and j == r // 4
                        nc.tensor.matmul(
                            P4[:, j * 512 : (j + 1) * 512],
                            lhsT,
                            fB[:, c * 512 : (c + 1) * 512],
                            start=True,
                            stop=not is_diag,
                        )
                        if is_diag:
                            off = (r % 4) * 128
                            nc.tensor.matmul(
                                P4[:, j * 512 : (j + 1) * 512],
                                bigi[:],
                                sel[:, 512 - off : 1024 - off],
                                start=False,
                                stop=True,
                            )
                    nc.vector.tensor_reduce(
                        mpart[:, r * 4 + q : r * 4 + q + 1],
                        P4[:, :].rearrange("p (f c) -> p f c", c=512),
                        axis=AX.XY,
                        op=ALU.min,
                    )

            # ---- finalize --------------------------------------------------
            m = wpool.tile([128, RT], F32, tag="m")
            nc.vector.tensor_reduce(
                m[:], mpart[:, :].rearrange("p (r q) -> p r q", q=4), axis=AX.X, op=ALU.min
            )
            mc = wpool.tile([128, RT], F32, tag="mc")
            nc.vector.tensor_scalar(mc[:], m[:], 0.0, None, op0=ALU.max)
            sn = wpool.tile([128, RT], F32, tag="sn")
            nc.scalar.activation(sn[:], mc[:], AF.Sqrt)
            valid = wpool.tile([128, RT], F32, tag="valid")
            nc.vector.tensor_scalar(valid[:], m[:], TH, None, op0=ALU.is_lt)
            dn = wpool.tile([128, RT], F32, tag="dn")
            nc.vector.tensor_tensor(dn[:], sn[:], valid[:], op=ALU.mult)
            pre = wpool.tile([128, RT], F32, tag="pre")
            nc.vector.tensor_tensor(pre[:], dpos[:], dn[:], op=ALU.subtract)
            losses = wpool.tile([128, RT], F32, tag="losses")
            nc.scalar.activation(losses[:], pre[:], AF.Relu, bias=MARGIN)
            nc.sync.dma_start(out=out[:, :], in_=losses[:])

    _split_drain_waits(nc)
    return nc


def _build_nc_sorted(gpc, padg):
    """Group-sorted variant: each core gets `gpc` whole groups, each padded to
    `padg` rows/cols. Only within-group blocks are computed (the masked min
    never needs cross-group pairs). Columns = the core's own rows, so the
    self-pair diagonal is at block-local positions (SEL matmul trick).
    Padded columns carry constant-feature 2+BIG -> always excluded."""
    import ml_dtypes

    assert padg <= 512 and padg % 128 == 0
    rmax = gpc * padg          # rows (and cols) per core
    rt = rmax // 128           # 128-row tiles per core
    tpg = padg // 128          # row tiles per group

    nc = bass.Bass()
    a_in = nc.dram_tensor("a", [128, rt * D], F32, kind="ExternalInput")
    b_in = nc.dram_tensor("b", [128, rt * D], F32, kind="ExternalInput")
    cv_in = nc.dram_tensor("cv", [2, rmax], MM_DT, kind="ExternalInput")
    out = nc.dram_tensor("losses", [128, rt], F32, kind="ExternalOutput")

    ident_np = np.eye(128, dtype=np.float32)
    sel_np = np.zeros((128, 1024), dtype=np.float32)
    sel_np[np.arange(128), 512 + np.arange(128)] = 1.0
    bigi_np = (BIG * np.eye(128)).astype(ml_dtypes.bfloat16)
    ident_d = nc.inline_tensor(ident_np, name="identc")
    sel_d = nc.inline_tensor(sel_np.astype(ml_dtypes.bfloat16), name="selc")
    bigi_d = nc.inline_tensor(bigi_np, name="bigic")

    with TileContext(nc) as tc:
        with (
            tc.tile_pool(name="const", bufs=1) as cpool,
            tc.tile_pool(name="work", bufs=1) as wpool,
        ):
            ident = cpool.tile([128, 128], F32, tag="ident")
            nc.sync.dma_start(out=ident[:], in_=ident_d[:, :])
            sel = cpool.tile([128, 1024], BF16, tag="sel")
            nc.sync.dma_start(out=sel[:], in_=sel_d[:, :])
            bigi = cpool.tile([128, 128], BF16, tag="bigi")
            nc.sync.dma_start(out=bigi[:], in_=bigi_d[:, :])

            tA = wpool.tile([128, rt * D], F32, tag="tA")
            nc.sync.dma_start(out=tA[:], in_=a_in[:, :])
            tB = wpool.tile([128, rt * D], F32, tag="tB")
            nc.sync.dma_start(out=tB[:], in_=b_in[:, :])

            fA = cpool.tile([33, rmax], MM_DT, tag="fA")
            fB = cpool.tile([33, rmax], MM_DT, tag="fB")
            nc.sync.dma_start(out=fA[32:33, :], in_=cv_in[0:1, :])
            nc.sync.dma_start(out=fB[32:33, :], in_=cv_in[1:2, :])

            # normalize A rows
            tA3 = tA[:, :].rearrange("p (t d) -> p t d", d=D)
            sqA = wpool.tile([128, rt * D], F32, tag="sqA")
            nc.scalar.activation(sqA[:], tA[:], AF.Square)
            ssA = wpool.tile([128, rt], F32, tag="ssA")
            nc.vector.tensor_reduce(
                ssA[:], sqA[:, :].rearrange("p (t d) -> p t d", d=D), axis=AX.X, op=ALU.add
            )
            nA = wpool.tile([128, rt], F32, tag="nA")
            nc.scalar.activation(nA[:], ssA[:], AF.Sqrt)
            rA = wpool.tile([128, rt], F32, tag="rA")
            nc.vector.reciprocal(rA[:], nA[:])
            an = wpool.tile([128, rt * D], F32, tag="an")
            an3 = an[:, :].rearrange("p (t d) -> p t d", d=D)
            nc.vector.tensor_tensor(
                an3, tA3, rA[:, :].broadcast_to([128, rt, D]), op=ALU.mult
            )

            # normalize B rows (and -2 scale for features)
            tB3 = tB[:, :].rearrange("p (t d) -> p t d", d=D)
            sqB = wpool.tile([128, rt * D], F32, tag="sqB")
            nc.scalar.activation(sqB[:], tB[:], AF.Square)
            ssB = wpool.tile([128, rt], F32, tag="ssB")
            nc.vector.tensor_reduce(
                ssB[:], sqB[:, :].rearrange("p (t d) -> p t d", d=D), axis=AX.X, op=ALU.add
            )
            nB = wpool.tile([128, rt], F32, tag="nB")
            nc.scalar.activation(nB[:], ssB[:], AF.Sqrt)
            rB = wpool.tile([128, rt], F32, tag="rB")
            nc.vector.reciprocal(rB[:], nB[:])
            rBm2 = wpool.tile([128, rt], F32, tag="rBm2")
            nc.vector.tensor_scalar(rBm2[:], rB[:], -2.0, None, op0=ALU.mult)
            bn2 = wpool.tile([128, rt * D], F32, tag="bn2")
            bn23 = bn2[:, :].rearrange("p (t d) -> p t d", d=D)
            nc.vector.tensor_tensor(
                bn23, tB3, rBm2[:, :].broadcast_to([128, rt, D]), op=ALU.mult
            )

            # dist_pos
            bno = wpool.tile([128, rt * D], F32, tag="bno")
            bno3 = bno[:, :].rearrange("p (t d) -> p t d", d=D)
            nc.vector.tensor_tensor(
                bno3, tB3, rB[:, :].broadcast_to([128, rt, D]), op=ALU.mult
            )
            dd = wpool.tile([128, rt * D], F32, tag="dd")
            nc.vector.tensor_tensor(dd[:], an[:], bno[:], op=ALU.subtract)
            sqd = wpool.tile([128, rt * D], F32, tag="sqd")
            nc.scalar.activation(sqd[:], dd[:], AF.Square)
            dp2 = wpool.tile([128, rt], F32, tag="dp2")
            nc.vector.tensor_reduce(
                dp2[:], sqd[:, :].rearrange("p (t d) -> p t d", d=D), axis=AX.X, op=ALU.add
            )
            dpos = wpool.tile([128, rt], F32, tag="dpos")
            nc.scalar.activation(dpos[:], dp2[:], AF.Sqrt)

            # transposes -> feature layout
            with tc.tile_pool(name="pst", bufs=2, space="PSUM") as pstp:
                psA = pstp.tile([32, rmax], F32, tag="pst")
                for t in range(rt):
                    nc.tensor.transpose(
                        psA[:, t * 128 : (t + 1) * 128], an3[:, t, :], ident[:]
                    )
                nc.scalar.copy(fA[0:32, :], psA[:, :])
                psB = pstp.tile([32, rmax], F32, tag="pst")
                for t in range(rt):
                    nc.tensor.transpose(
                        psB[:, t * 128 : (t + 1) * 128], bn23[:, t, :], ident[:]
                    )
                nc.scalar.copy(fB[0:32, :], psB[:, :])

            # per-group fused matmul + diag fix + min reduce
            mpart = wpool.tile([128, rt], F32, tag="mpart")
            with tc.tile_pool(name="psm", bufs=4, space="PSUM") as psmp:
                for gl in range(gpc):
                    for r in range(tpg):
                        idx = gl * tpg + r
                        off = r * 128
                        P = psmp.tile([128, 512], F32, tag="psm")
                        nc.tensor.matmul(
                            P[:, :padg],
                            fA[:, idx * 128 : (idx + 1) * 128],
                            fB[:, gl * padg : (gl + 1) * padg],
                            start=True,
                            stop=False,
                        )
                        nc.tensor.matmul(
                            P[:, :padg],
                            bigi[:],
                            sel[:, 512 - off : 512 - off + padg],
                            start=False,
                            stop=True,
                        )
                        nc.vector.tensor_reduce(
                            mpart[:, idx : idx + 1], P[:, :padg], axis=AX.X, op=ALU.min
                        )

            # finalize
            mc = wpool.tile([128, rt], F32, tag="mc")
            nc.vector.tensor_scalar(mc[:], mpart[:], 0.0, None, op0=ALU.max)
            sn = wpool.tile([128, rt], F32, tag="sn")
            nc.scalar.activation(sn[:], mc[:], AF.Sqrt)
            valid = wpool.tile([128, rt], F32, tag="valid")
            nc.vector.tensor_scalar(valid[:], mpart[:], TH, None, op0=ALU.is_lt)
            dn = wpool.tile([128, rt], F32, tag="dn")
            nc.vector.tensor_tensor(dn[:], sn[:], valid[:], op=ALU.mult)
            pre = wpool.tile([128, rt], F32, tag="pre")
            nc.vector.tensor_tensor(pre[:], dpos[:], dn[:], op=ALU.subtract)
            losses = wpool.tile([128, rt], F32, tag="losses")
            nc.scalar.activation(losses[:], pre[:], AF.Relu, bias=MARGIN)
            nc.sync.dma_start(out=out[:, :], in_=losses[:])

    _split_drain_waits(nc)
    return nc


_NC_CACHE = None
_NC_SORTED_CACHE = {}


def _get_nc():
    global _NC_CACHE
    if _NC_CACHE is None:
        _NC_CACHE = _build_nc()
    return _NC_CACHE


def _get_nc_sorted(gpc, padg):
    key = (gpc, padg)
    if key not in _NC_SORTED_CACHE:
        _NC_SORTED_CACHE[key] = _build_nc_sorted(gpc, padg)
    return _NC_SORTED_CACHE[key]


def _tile_tp(x):
    """[R, 32] rows -> [128, (R/128)*32] with row t*128+p on partition p."""
    r = x.shape[0]
    return (
        np.ascontiguousarray(
            x.reshape(r // 128, 128, D).transpose(1, 0, 2).reshape(128, (r // 128) * D)
        )
    )


def _kernel_sorted(A, B, lab):
    counts = np.bincount(lab, minlength=G)
    gn = len(counts)
    gpc = -(-gn // NCORES)
    padg = max(128, -(-int(counts.max()) // 128) * 128)
    if padg > 512:
        return None  # degenerate label distribution: fall back to full kernel
    rmax = gpc * padg
    rt = rmax // 128

    order = np.argsort(lab, kind="stable")
    starts = np.concatenate([[0], np.cumsum(counts)])

    src = np.full((NCORES, rmax), -1, np.int64)
    for g in range(gn):
        c, gl = divmod(g, gpc)
        n = int(counts[g])
        src[c, gl * padg : gl * padg + n] = order[starts[g] : starts[g] + n]

    in_maps = []
    for c in range(NCORES):
        idx = src[c]
        real = idx >= 0
        a_rows = np.ones((rmax, D), np.float32)
        b_rows = np.ones((rmax, D), np.float32)
        a_rows[real] = A[idx[real]]
        b_rows[real] = B[idx[real]]
        cv = np.ones((2, rmax), np.float32)
        cv[1] = np.where(real, 2.0, 2.0 + BIG)
        in_maps.append(
            {
                "a": _tile_tp(a_rows),
                "b": _tile_tp(b_rows),
                "cv": np.ascontiguousarray(cv),
            }
        )

    global _last_in_maps, _last_nc
    _last_in_maps = in_maps
    nc = _get_nc_sorted(gpc, padg)
    _last_nc = nc
    res = run_bass_kernel_spmd(nc, in_maps, list(range(NCORES)))
    total = 0.0
    for c in range(NCORES):
        lo = res.results[c]["losses"]  # [128, rt]; [p, t] = loss of local row t*128+p
        flat = lo.T.reshape(rmax)
        real = src[c] >= 0
        total += float(flat[real].sum(dtype=np.float64))
    return np.float32(total / N)


def kernel(A=None, B=None, labels=None, **_unused):
    import os

    A = np.asarray(A, dtype=np.float32)
    B = np.asarray(B, dtype=np.float32)
    lab = np.asarray(labels).astype(np.int64)
    lab = lab - lab.min() if lab.min() < 0 else lab

    if not os.environ.get("KERNEL_FORCE_FULL"):
        out = _kernel_sorted(A, B, lab.astype(np.int32))
        if out is not None:
            return out

    eye = np.arange(G, dtype=np.int32)
    in_maps = []
    for c in range(NCORES):
        rows = slice(c * RPC, (c + 1) * RPC)
        a_c = _tile_tp(A[rows])
        b_rot = np.roll(B, -c * RPC, axis=0)
        lab_rot = np.roll(lab, -c * RPC)
        b_c = _tile_tp(b_rot)
        oha = np.concatenate(
            [
                np.ones((1, RPC), np.float32),
                (-BIG) * (lab[rows][None, :] == eye[:, None]).astype(np.float32),
            ]
        )
        ohb = np.concatenate(
            [
                np.full((1, N), 2.0 + BIG, np.float32),
                (lab_rot[None, :] == eye[:, None]).astype(np.float32),
            ]
        )
        in_maps.append(
            {
                "a": a_c,
                "b": b_c,
                "oha": np.ascontiguousarray(oha),
                "ohb": np.ascontiguousarray(ohb),
            }
        )

    global _last_in_maps, _last_nc
    _last_in_maps = in_maps
    nc = _get_nc()
    _last_nc = nc
    res = run_bass_kernel_spmd(nc, in_maps, list(range(NCORES)))
    total = 0.0
    for c in range(NCORES):
        lo = res.results[c]["losses"]  # [128, RT]; [p, r] = loss of row r*128+p
        total += float(lo.sum(dtype=np.float64))
    return np.float32(total / N)


# revision 19
# speedup vs baseline: 4.4753x; 1.0559x over previous
"""Grouped triplet loss on 8 trn2 NeuronCores.

Strategy (data-parallel over A rows, hint-compliant):
  - Each core takes a 1024-row block of A, full B (column-rotated so the
    diagonal of the distance matrix lands at core-independent positions).
  - L2 normalization of A-block and B on device.
  - One fused matmul per (row-tile, col-chunk) computes the *masked* squared
    distance directly in PSUM via extended feature vectors:
        F_A = [ a_i (32) | 1 | -BIG*onehot(label_i) (32) ]   (K = 65)
        F_B = [ -2*b_j   | 2+BIG |      onehot(label_j)  ]
    so PSUM = 2 - 2*a.b + BIG*(1 - same_group).
  - A tiny bf16 identity matmul accumulates +BIG on the diagonal (self-pair
    exclusion).
  - DVE min-reduces PSUM (4 banks per op); rows with min >= TH had no valid
    negative -> dist_neg = 0 (matches torch "skip groups of size < 2").
  - losses = relu(dist_pos - dist_neg + margin); host averages.

Host-side work is limited to sharding/layout: slicing, row-rotation, (t p)
tiling, and one-hot encoding of the integer labels. All float math happens
on device.
"""

import numpy as np

import concourse.bass as bass
import concourse.mybir as mybir
from concourse.tile import TileContext
from concourse.bass_utils import run_bass_kernel_spmd

N, D, G = 8192, 32, 32
NCORES = 8
RPC = N // NCORES      # rows per core = 1024
RT = RPC // 128        # row tiles per core = 8
CT = N // 128          # column tiles = 64
NCHUNK = N // 512      # matmul column chunks = 16
BIG = 64.0
TH = 32.0
MARGIN = 1.0

F32 = mybir.dt.float32
BF16 = mybir.dt.bfloat16
AF = mybir.ActivationFunctionType
ALU = mybir.AluOpType
AX = mybir.AxisListType

MM_DT = mybir.dt.float32r  # matmul feature dtype (float32 | float32r)

_MAX_DRAIN_WAITS = 1


def _split_drain_waits(nc):
    """This container's walrus rejects any instruction with >1 sem-wait.
    Hoist excess waits onto preceding same-engine single-wait Drains."""
    nsplit = 0
    for f in nc.m.functions:
        for bb in f.blocks:
            new_insts = []
            for inst in bb.instructions:
                si = inst.sync_info
                waits = list(si.on_wait) if si and si.on_wait else []
                if len(waits) > _MAX_DRAIN_WAITS:
                    extra, keep = waits[:-_MAX_DRAIN_WAITS], waits[-_MAX_DRAIN_WAITS:]
                    for w in extra:
                        d = mybir.InstDrain(
                            name=f"{inst.name}-swsplit{nsplit}",
                            engine=inst.engine,
                            ins=[],
                            outs=[],
                            sync_info=mybir.SyncInfo(on_wait=[w], on_update=[]),
                        )
                        nsplit += 1
                        nc.register_instruction(d, overwrite=True)
                        new_insts.append(d)
                    si.on_wait = keep
                new_insts.append(inst)
            bb.instructions[:] = new_insts


def _build_nc():
    import ml_dtypes

    nc = bass.Bass()

    a_in = nc.dram_tensor("a", [128, RT * D], F32, kind="ExternalInput")
    b_in = nc.dram_tensor("b", [128, CT * D], F32, kind="ExternalInput")
    # row 0: constant feature (1 for A, 2+BIG for B); rows 1..32: one-hot
    oha_in = nc.dram_tensor("oha", [G + 1, RPC], MM_DT, kind="ExternalInput")
    ohb_in = nc.dram_tensor("ohb", [G + 1, N], MM_DT, kind="ExternalInput")
    out = nc.dram_tensor("losses", [128, RT], F32, kind="ExternalOutput")

    ident_np = np.eye(128, dtype=np.float32)
    sel_np = np.zeros((128, 1024), dtype=np.float32)
    sel_np[np.arange(128), 512 + np.arange(128)] = 1.0
    bigi_np = (BIG * np.eye(128)).astype(ml_dtypes.bfloat16)
    ident_d = nc.inline_tensor(ident_np, name="identc")
    sel_d = nc.inline_tensor(sel_np.astype(ml_dtypes.bfloat16), name="selc")
    bigi_d = nc.inline_tensor(bigi_np, name="bigic")

    with TileContext(nc) as tc:
        with (
            tc.tile_pool(name="const", bufs=1) as cpool,
            tc.tile_pool(name="work", bufs=1) as wpool,
            tc.tile_pool(name="ps", bufs=2, space="PSUM") as pspool,
        ):
            # ---- constants -------------------------------------------------
            ident = cpool.tile([128, 128], F32, tag="ident")
            nc.sync.dma_start(out=ident[:], in_=ident_d[:, :])
            sel = cpool.tile([128, 1024], BF16, tag="sel")
            nc.sync.dma_start(out=sel[:], in_=sel_d[:, :])
            bigi = cpool.tile([128, 128], BF16, tag="bigi")
            nc.sync.dma_start(out=bigi[:], in_=bigi_d[:, :])

            # ---- raw loads -------------------------------------------------
            tA = wpool.tile([128, RT * D], F32, tag="tA")
            nc.sync.dma_start(out=tA[:], in_=a_in[:, :])
            tB = wpool.tile([128, CT * D], F32, tag="tB")
            # split into 2 DMAs to use more queues
            nc.sync.dma_start(out=tB[:, : CT * D // 2], in_=b_in[:, : CT * D // 2])
            nc.sync.dma_start(out=tB[:, CT * D // 2 :], in_=b_in[:, CT * D // 2 :])

            fA = cpool.tile([G + 33, RPC], MM_DT, tag="fA")
            fB = cpool.tile([G + 33, N], MM_DT, tag="fB")
            nc.sync.dma_start(out=fA[32:65, :], in_=oha_in[:, :])
            nc.sync.dma_start(out=fB[32:65, : N // 2], in_=ohb_in[:, : N // 2])
            nc.sync.dma_start(out=fB[32:65, N // 2 :], in_=ohb_in[:, N // 2 :])

            # ---- normalize A block ----------------------------------------
            tA3 = tA[:, :].rearrange("p (t d) -> p t d", d=D)
            sqA = wpool.tile([128, RT * D], F32, tag="sqA")
            nc.scalar.activation(sqA[:], tA[:], AF.Square)
            ssA = wpool.tile([128, RT], F32, tag="ssA")
            nc.vector.tensor_reduce(
                ssA[:], sqA[:, :].rearrange("p (t d) -> p t d", d=D), axis=AX.X, op=ALU.add
            )
            nA = wpool.tile([128, RT], F32, tag="nA")
            nc.scalar.activation(nA[:], ssA[:], AF.Sqrt)
            rA = wpool.tile([128, RT], F32, tag="rA")
            nc.vector.reciprocal(rA[:], nA[:])
            an = wpool.tile([128, RT * D], F32, tag="an")
            an3 = an[:, :].rearrange("p (t d) -> p t d", d=D)
            nc.vector.tensor_tensor(
                an3, tA3, rA[:, :].broadcast_to([128, RT, D]), op=ALU.mult
            )

            # ---- normalize B (scaled by -2 for features) -------------------
            tB3 = tB[:, :].rearrange("p (t d) -> p t d", d=D)
            sqB = wpool.tile([128, CT * D], F32, tag="sqB")
            nc.scalar.activation(sqB[:], tB[:], AF.Square)
            ssB = wpool.tile([128, CT], F32, tag="ssB")
            nc.vector.tensor_reduce(
                ssB[:], sqB[:, :].rearrange("p (t d) -> p t d", d=D), axis=AX.X, op=ALU.add
            )
            nB = wpool.tile([128, CT], F32, tag="nB")
            nc.scalar.activation(nB[:], ssB[:], AF.Sqrt)
            rB = wpool.tile([128, CT], F32, tag="rB")
            nc.vector.reciprocal(rB[:], nB[:])
            rBm2 = wpool.tile([128, CT], F32, tag="rBm2")
            nc.vector.tensor_scalar(rBm2[:], rB[:], -2.0, None, op0=ALU.mult)
            bn2 = wpool.tile([128, CT * D], F32, tag="bn2")
            bn23 = bn2[:, :].rearrange("p (t d) -> p t d", d=D)
            nc.vector.tensor_tensor(
                bn23, tB3, rBm2[:, :].broadcast_to([128, CT, D]), op=ALU.mult
            )

            # ---- transpose an -> fA[0:32, :] ------------------------------
            psA = pspool.tile([32, RPC], F32, tag="ps")
            for r in range(RT):
                nc.tensor.transpose(psA[:, r * 128 : (r + 1) * 128], an3[:, r, :], ident[:])
            nc.scalar.copy(fA[0:32, :], psA[:, :])

            # ---- transpose bn2 -> fB[0:32, :] ------------------------------
            for grp in range(CT // 16):
                psB = pspool.tile([32, 16 * 128], F32, tag="ps")
                for k in range(16):
                    t = grp * 16 + k
                    nc.tensor.transpose(
                        psB[:, k * 128 : (k + 1) * 128], bn23[:, t, :], ident[:]
                    )
                nc.scalar.copy(fB[0:32, grp * 2048 : (grp + 1) * 2048], psB[:, :])

            # ---- dist_pos for own rows (first RT tiles of rotated B) ------
            bno = wpool.tile([128, RT * D], F32, tag="bno")
            bno3 = bno[:, :].rearrange("p (t d) -> p t d", d=D)
            nc.vector.tensor_tensor(
                bno3, tB3[:, 0:RT, :], rB[:, 0:RT].broadcast_to([128, RT, D]), op=ALU.mult
            )
            dd = wpool.tile([128, RT * D], F32, tag="dd")
            nc.vector.tensor_tensor(dd[:], an[:], bno[:], op=ALU.subtract)
            sqd = wpool.tile([128, RT * D], F32, tag="sqd")
            nc.scalar.activation(sqd[:], dd[:], AF.Square)
            dp2 = wpool.tile([128, RT], F32, tag="dp2")
            nc.vector.tensor_reduce(
                dp2[:], sqd[:, :].rearrange("p (t d) -> p t d", d=D), axis=AX.X, op=ALU.add
            )
            dpos = wpool.tile([128, RT], F32, tag="dpos")
            nc.scalar.activation(dpos[:], dp2[:], AF.Sqrt)

            # ---- main loop: fused matmul + masked min ----------------------
            mpart = wpool.tile([128, RT * 4], F32, tag="mpart")
            for r in range(RT):
                lhsT = fA[:, r * 128 : (r + 1) * 128]
                for q in range(4):
                    P4 = pspool.tile([128, 2048], F32, tag="ps")
                    for j in range(4):
                        c = q * 4 + j
                        is_diag = q == 0 and j == r // 4
                        nc.tensor.matmul(
                            P4[:, j * 512 : (j + 1) * 512],
                            lhsT,
                            fB[:, c * 512 : (c + 1) * 512],
                            start=True,
                            stop=not is_diag,
                        )
                        if is_diag:
                            off = (r % 4) * 128
                            nc.tensor.matmul(
                                P4[:, j * 512 : (j + 1) * 512],
                                bigi[:],
                                sel[:, 512 - off : 1024 - off],
                                start=False,
                                stop=True,
                            )
                    nc.vector.tensor_reduce(
                        mpart[:, r * 4 + q : r * 4 + q + 1],
                        P4[:, :].rearrange("p (f c) -> p f c", c=512),
                        axis=AX.XY,
                        op=ALU.min,
                    )

            # ---- finalize --------------------------------------------------
            m = wpool.tile([128, RT], F32, tag="m")
            nc.vector.tensor_reduce(
                m[:], mpart[:, :].rearrange("p (r q) -> p r q", q=4), axis=AX.X, op=ALU.min
            )
            mc = wpool.tile([128, RT], F32, tag="mc")
            nc.vector.tensor_scalar(mc[:], m[:], 0.0, None, op0=ALU.max)
            sn = wpool.tile([128, RT], F32, tag="sn")
            nc.scalar.activation(sn[:], mc[:], AF.Sqrt)
            valid = wpool.tile([128, RT], F32, tag="valid")
            nc.vector.tensor_scalar(valid[:], m[:], TH, None, op0=ALU.is_lt)
            dn = wpool.tile([128, RT], F32, tag="dn")
            nc.vector.tensor_tensor(dn[:], sn[:], valid[:], op=ALU.mult)
            pre = wpool.tile([128, RT], F32, tag="pre")
            nc.vector.tensor_tensor(pre[:], dpos[:], dn[:], op=ALU.subtract)
            losses = wpool.tile([128, RT], F32, tag="losses")
            nc.scalar.activation(losses[:], pre[:], AF.Relu, bias=MARGIN)
            nc.sync.dma_start(out=out[:, :], in_=losses[:])

    _split_drain_waits(nc)
    return nc


def _build_nc_sorted(gpc, padg):
    """Group-sorted variant: each core gets `gpc` whole groups, each padded to
    `padg` rows/cols. Only within-group blocks are computed (the masked min
    never needs cross-group pairs). Columns = the core's own rows, so the
    self-pair diagonal is at block-local positions; it is excluded by fusing
    "+BIG on the diagonal" into the min-reduce (tensor_tensor_reduce op0=add
    with a shifted-diagonal constant). Padded columns carry constant-feature
    2+BIG -> always excluded."""
    assert padg <= 512 and padg % 128 == 0
    rmax = gpc * padg          # rows (and cols) per core
    rt = rmax // 128           # 128-row tiles per core
    tpg = padg // 128          # row tiles per group

    nc = bass.Bass()
    a_in = nc.dram_tensor("a", [128, rt * D], F32, kind="ExternalInput")
    b_in = nc.dram_tensor("b", [128, rt * D], F32, kind="ExternalInput")
    cv_in = nc.dram_tensor("cv", [2, rmax], MM_DT, kind="ExternalInput")
    out = nc.dram_tensor("losses", [128, rt], F32, kind="ExternalOutput")

    ident_np = np.eye(128, dtype=np.float32)
    seld_np = (BIG * np.eye(128)).astype(np.float32)
    ident_d = nc.inline_tensor(ident_np, name="identc")
    seld_d = nc.inline_tensor(seld_np, name="seldc")

    with TileContext(nc) as tc:
        with (
            tc.tile_pool(name="const", bufs=1) as cpool,
            tc.tile_pool(name="work", bufs=1) as wpool,
        ):
            ident = cpool.tile([128, 128], F32, tag="ident")
            nc.sync.dma_start(out=ident[:], in_=ident_d[:, :])
            seld = cpool.tile([128, 128], F32, tag="seld")
            nc.sync.dma_start(out=seld[:], in_=seld_d[:, :])

            # A rows and B rows in one tile: one normalization chain
            tAB = wpool.tile([128, 2 * rt * D], F32, tag="tAB")
            nc.sync.dma_start(out=tAB[:, : rt * D], in_=a_in[:, :])
            nc.sync.dma_start(out=tAB[:, rt * D :], in_=b_in[:, :])
            tAB3 = tAB[:, :].rearrange("p (t d) -> p t d", d=D)

            fA = cpool.tile([33, rmax], MM_DT, tag="fA")
            fB = cpool.tile([33, rmax], MM_DT, tag="fB")
            nc.sync.dma_start(out=fA[32:33, :], in_=cv_in[0:1, :])
            nc.sync.dma_start(out=fB[32:33, :], in_=cv_in[1:2, :])

            # ---- joint normalization of A and B rows ----
            sqAB = wpool.tile([128, 2 * rt * D], F32, tag="sqAB")
            nc.scalar.activation(sqAB[:], tAB[:], AF.Square)
            ssAB = wpool.tile([128, 2 * rt], F32, tag="ssAB")
            nc.vector.tensor_reduce(
                ssAB[:], sqAB[:, :].rearrange("p (t d) -> p t d", d=D), axis=AX.X, op=ALU.add
            )
            nAB = wpool.tile([128, 2 * rt], F32, tag="nAB")
            nc.scalar.activation(nAB[:], ssAB[:], AF.Sqrt)
            rAB = wpool.tile([128, 2 * rt], F32, tag="rAB")
            nc.vector.reciprocal(rAB[:], nAB[:])
            # scale factors: rA for an, -2*rB for bn2
            sc = wpool.tile([128, 2 * rt], F32, tag="sc")
            nc.vector.tensor_copy(sc[:, :rt], rAB[:, :rt])
            nc.vector.tensor_scalar(sc[:, rt:], rAB[:, rt:], -2.0, None, op0=ALU.mult)
            anbn = wpool.tile([128, 2 * rt * D], F32, tag="anbn")
            anbn3 = anbn[:, :].rearrange("p (t d) -> p t d", d=D)
            nc.vector.tensor_tensor(
                anbn3, tAB3, sc[:, :].broadcast_to([128, 2 * rt, D]), op=ALU.mult
            )
            an3 = anbn3[:, 0:rt, :]
            bn23 = anbn3[:, rt : 2 * rt, :]

            # ---- dist_pos: || an - bn || = || an + 0.5*bn2 || ----
            bno = wpool.tile([128, rt * D], F32, tag="bno")
            nc.vector.tensor_scalar(
                bno[:, :].rearrange("p (t d) -> p t d", d=D), bn23, -0.5, None, op0=ALU.mult
            )
            dd = wpool.tile([128, rt * D], F32, tag="dd")
            nc.vector.tensor_tensor(
                dd[:, :].rearrange("p (t d) -> p t d", d=D), an3, bno[:, :].rearrange("p (t d) -> p t d", d=D), op=ALU.subtract
            )
            sqd = wpool.tile([128, rt * D], F32, tag="sqd")
            nc.scalar.activation(sqd[:], dd[:], AF.Square)
            dp2 = wpool.tile([128, rt], F32, tag="dp2")
            nc.vector.tensor_reduce(
                dp2[:], sqd[:, :].rearrange("p (t d) -> p t d", d=D), axis=AX.X, op=ALU.add
            )
            dpos = wpool.tile([128, rt], F32, tag="dpos")
            nc.scalar.activation(dpos[:], dp2[:], AF.Sqrt)

            # ---- transposes -> feature layout ----
            with tc.tile_pool(name="pst", bufs=2, space="PSUM") as pstp:
                psA = pstp.tile([32, rmax], F32, tag="pst")
                for t in range(rt):
                    nc.tensor.transpose(
                        psA[:, t * 128 : (t + 1) * 128], an3[:, t, :], ident[:]
                    )
                nc.scalar.copy(fA[0:32, :], psA[:, :])
                psB = pstp.tile([32, rmax], F32, tag="pst")
                for t in range(rt):
                    nc.tensor.transpose(
                        psB[:, t * 128 : (t + 1) * 128], bn23[:, t, :], ident[:]
                    )
                nc.scalar.copy(fB[0:32, :], psB[:, :])

            # ---- per-group matmul + fused (diag-add + min) reduce ----
            mpart = wpool.tile([128, rt], F32, tag="mpart")
            with tc.tile_pool(name="psm", bufs=4, space="PSUM") as psmp:
                for gl in range(gpc):
                    for r in range(tpg):
                        idx = gl * tpg + r
                        off = r * 128
                        P = psmp.tile([128, 512], F32, tag="psm")
                        nc.tensor.matmul(
                            P[:, :padg],
                            fA[:, idx * 128 : (idx + 1) * 128],
                            fB[:, gl * padg : (gl + 1) * padg],
                            start=True,
                            stop=True,
                        )
                        # exclude the self-pair: +BIG on the block diagonal
                        nc.vector.tensor_tensor(
                            P[:, off : off + 128], P[:, off : off + 128], seld[:], op=ALU.add
                        )
                        nc.vector.tensor_reduce(
                            mpart[:, idx : idx + 1], P[:, :padg], axis=AX.X, op=ALU.min
                        )

            # finalize
            mc = wpool.tile([128, rt], F32, tag="mc")
            nc.vector.tensor_scalar(mc[:], mpart[:], 0.0, None, op0=ALU.max)
            sn = wpool.tile([128, rt], F32, tag="sn")
            nc.scalar.activation(sn[:], mc[:], AF.Sqrt)
            valid = wpool.tile([128, rt], F32, tag="valid")
            nc.vector.tensor_scalar(valid[:], mpart[:], TH, None, op0=ALU.is_lt)
            dn = wpool.tile([128, rt], F32, tag="dn")
            nc.vector.tensor_tensor(dn[:], sn[:], valid[:], op=ALU.mult)
            pre = wpool.tile([128, rt], F32, tag="pre")
            nc.vector.tensor_tensor(pre[:], dpos[:], dn[:], op=ALU.subtract)
            losses = wpool.tile([128, rt], F32, tag="losses")
            nc.scalar.activation(losses[:], pre[:], AF.Relu, bias=MARGIN)
            nc.sync.dma_start(out=out[:, :], in_=losses[:])

    _split_drain_waits(nc)
    return nc


_NC_CACHE = None
_NC_SORTED_CACHE = {}


def _get_nc():
    global _NC_CACHE
    if _NC_CACHE is None:
        _NC_CACHE = _build_nc()
    return _NC_CACHE


def _get_nc_sorted(gpc, padg):
    key = (gpc, padg)
    if key not in _NC_SORTED_CACHE:
        _NC_SORTED_CACHE[key] = _build_nc_sorted(gpc, padg)
    return _NC_SORTED_CACHE[key]


def _tile_tp(x):
    """[R, 32] rows -> [128, (R/128)*32] with row t*128+p on partition p."""
    r = x.shape[0]
    return (
        np.ascontiguousarray(
            x.reshape(r // 128, 128, D).transpose(1, 0, 2).reshape(128, (r // 128) * D)
        )
    )


def _kernel_sorted(A, B, lab):
    counts = np.bincount(lab, minlength=G)
    gn = len(counts)
    gpc = -(-gn // NCORES)
    padg = max(128, -(-int(counts.max()) // 128) * 128)
    if padg > 512:
        return None  # degenerate label distribution: fall back to full kernel
    rmax = gpc * padg
    rt = rmax // 128

    order = np.argsort(lab, kind="stable")
    starts = np.concatenate([[0], np.cumsum(counts)])

    src = np.full((NCORES, rmax), -1, np.int64)
    for g in range(gn):
        c, gl = divmod(g, gpc)
        n = int(counts[g])
        src[c, gl * padg : gl * padg + n] = order[starts[g] : starts[g] + n]

    in_maps = []
    for c in range(NCORES):
        idx = src[c]
        real = idx >= 0
        a_rows = np.ones((rmax, D), np.float32)
        b_rows = np.ones((rmax, D), np.float32)
        a_rows[real] = A[idx[real]]
        b_rows[real] = B[idx[real]]
        cv = np.ones((2, rmax), np.float32)
        cv[1] = np.where(real, 2.0, 2.0 + BIG)
        in_maps.append(
            {
                "a": _tile_tp(a_rows),
                "b": _tile_tp(b_rows),
                "cv": np.ascontiguousarray(cv),
            }
        )

    global _last_in_maps, _last_nc
    _last_in_maps = in_maps
    nc = _get_nc_sorted(gpc, padg)
    _last_nc = nc
    res = run_bass_kernel_spmd(nc, in_maps, list(range(NCORES)))
    total = 0.0
    for c in range(NCORES):
        lo = res.results[c]["losses"]  # [128, rt]; [p, t] = loss of local row t*128+p
        flat = lo.T.reshape(rmax)
        real = src[c] >= 0
        total += float(flat[real].sum(dtype=np.float64))
    return np.float32(total / N)


def kernel(A=None, B=None, labels=None, **_unused):
    import os

    A = np.asarray(A, dtype=np.float32)
    B = np.asarray(B, dtype=np.float32)
    lab = np.asarray(labels).astype(np.int64)
    lab = lab - lab.min() if lab.min() < 0 else lab

    if not os.environ.get("KERNEL_FORCE_FULL"):
        out = _kernel_sorted(A, B, lab.astype(np.int32))
        if out is not None:
            return out

    eye = np.arange(G, dtype=np.int32)
    in_maps = []
    for c in range(NCORES):
        rows = slice(c * RPC, (c + 1) * RPC)
        a_c = _tile_tp(A[rows])
        b_rot = np.roll(B, -c * RPC, axis=0)
        lab_rot = np.roll(lab, -c * RPC)
        b_c = _tile_tp(b_rot)
        oha = np.concatenate(
            [
                np.ones((1, RPC), np.float32),
                (-BIG) * (lab[rows][None, :] == eye[:, None]).astype(np.float32),
            ]
        )
        ohb = np.concatenate(
            [
                np.full((1, N), 2.0 + BIG, np.float32),
                (lab_rot[None, :] == eye[:, None]).astype(np.float32),
            ]
        )
        in_maps.append(
            {
                "a": a_c,
                "b": b_c,
                "oha": np.ascontiguousarray(oha),
                "ohb": np.ascontiguousarray(ohb),
            }
        )

    global _last_in_maps, _last_nc
    _last_in_maps = in_maps
    nc = _get_nc()
    _last_nc = nc
    res = run_bass_kernel_spmd(nc, in_maps, list(range(NCORES)))
    total = 0.0
    for c in range(NCORES):
        lo = res.results[c]["losses"]  # [128, RT]; [p, r] = loss of row r*128+p
        total += float(lo.sum(dtype=np.float64))
    return np.float32(total / N)


# revision 20
# speedup vs baseline: 4.5537x; 1.0175x over previous
"""Grouped triplet loss on 8 trn2 NeuronCores.

Strategy (data-parallel over A rows, hint-compliant):
  - Each core takes a 1024-row block of A, full B (column-rotated so the
    diagonal of the distance matrix lands at core-independent positions).
  - L2 normalization of A-block and B on device.
  - One fused matmul per (row-tile, col-chunk) computes the *masked* squared
    distance directly in PSUM via extended feature vectors:
        F_A = [ a_i (32) | 1 | -BIG*onehot(label_i) (32) ]   (K = 65)
        F_B = [ -2*b_j   | 2+BIG |      onehot(label_j)  ]
    so PSUM = 2 - 2*a.b + BIG*(1 - same_group).
  - A tiny bf16 identity matmul accumulates +BIG on the diagonal (self-pair
    exclusion).
  - DVE min-reduces PSUM (4 banks per op); rows with min >= TH had no valid
    negative -> dist_neg = 0 (matches torch "skip groups of size < 2").
  - losses = relu(dist_pos - dist_neg + margin); host averages.

Host-side work is limited to sharding/layout: slicing, row-rotation, (t p)
tiling, and one-hot encoding of the integer labels. All float math happens
on device.
"""

import numpy as np

import concourse.bass as bass
import concourse.mybir as mybir
from concourse.tile import TileContext
from concourse.bass_utils import run_bass_kernel_spmd

N, D, G = 8192, 32, 32
NCORES = 8
RPC = N // NCORES      # rows per core = 1024
RT = RPC // 128        # row tiles per core = 8
CT = N // 128          # column tiles = 64
NCHUNK = N // 512      # matmul column chunks = 16
BIG = 64.0
TH = 32.0
MARGIN = 1.0

F32 = mybir.dt.float32
BF16 = mybir.dt.bfloat16
AF = mybir.ActivationFunctionType
ALU = mybir.AluOpType
AX = mybir.AxisListType

MM_DT = mybir.dt.float32r  # matmul feature dtype (float32 | float32r)

_MAX_DRAIN_WAITS = 1


def _split_drain_waits(nc):
    """This container's walrus rejects any instruction with >1 sem-wait.
    Hoist excess waits onto preceding same-engine single-wait Drains."""
    nsplit = 0
    for f in nc.m.functions:
        for bb in f.blocks:
            new_insts = []
            for inst in bb.instructions:
                si = inst.sync_info
                waits = list(si.on_wait) if si and si.on_wait else []
                if len(waits) > _MAX_DRAIN_WAITS:
                    extra, keep = waits[:-_MAX_DRAIN_WAITS], waits[-_MAX_DRAIN_WAITS:]
                    for w in extra:
                        d = mybir.InstDrain(
                            name=f"{inst.name}-swsplit{nsplit}",
                            engine=inst.engine,
                            ins=[],
                            outs=[],
                            sync_info=mybir.SyncInfo(on_wait=[w], on_update=[]),
                        )
                        nsplit += 1
                        nc.register_instruction(d, overwrite=True)
                        new_insts.append(d)
                    si.on_wait = keep
                new_insts.append(inst)
            bb.instructions[:] = new_insts


def _build_nc():
    import ml_dtypes

    nc = bass.Bass()

    a_in = nc.dram_tensor("a", [128, RT * D], F32, kind="ExternalInput")
    b_in = nc.dram_tensor("b", [128, CT * D], F32, kind="ExternalInput")
    # row 0: constant feature (1 for A, 2+BIG for B); rows 1..32: one-hot
    oha_in = nc.dram_tensor("oha", [G + 1, RPC], MM_DT, kind="ExternalInput")
    ohb_in = nc.dram_tensor("ohb", [G + 1, N], MM_DT, kind="ExternalInput")
    out = nc.dram_tensor("losses", [128, RT], F32, kind="ExternalOutput")

    ident_np = np.eye(128, dtype=np.float32)
    sel_np = np.zeros((128, 1024), dtype=np.float32)
    sel_np[np.arange(128), 512 + np.arange(128)] = 1.0
    bigi_np = (BIG * np.eye(128)).astype(ml_dtypes.bfloat16)
    ident_d = nc.inline_tensor(ident_np, name="identc")
    sel_d = nc.inline_tensor(sel_np.astype(ml_dtypes.bfloat16), name="selc")
    bigi_d = nc.inline_tensor(bigi_np, name="bigic")

    with TileContext(nc) as tc:
        with (
            tc.tile_pool(name="const", bufs=1) as cpool,
            tc.tile_pool(name="work", bufs=1) as wpool,
            tc.tile_pool(name="ps", bufs=2, space="PSUM") as pspool,
        ):
            # ---- constants -------------------------------------------------
            ident = cpool.tile([128, 128], F32, tag="ident")
            nc.sync.dma_start(out=ident[:], in_=ident_d[:, :])
            sel = cpool.tile([128, 1024], BF16, tag="sel")
            nc.sync.dma_start(out=sel[:], in_=sel_d[:, :])
            bigi = cpool.tile([128, 128], BF16, tag="bigi")
            nc.sync.dma_start(out=bigi[:], in_=bigi_d[:, :])

            # ---- raw loads -------------------------------------------------
            tA = wpool.tile([128, RT * D], F32, tag="tA")
            nc.sync.dma_start(out=tA[:], in_=a_in[:, :])
            tB = wpool.tile([128, CT * D], F32, tag="tB")
            # split into 2 DMAs to use more queues
            nc.sync.dma_start(out=tB[:, : CT * D // 2], in_=b_in[:, : CT * D // 2])
            nc.sync.dma_start(out=tB[:, CT * D // 2 :], in_=b_in[:, CT * D // 2 :])

            fA = cpool.tile([G + 33, RPC], MM_DT, tag="fA")
            fB = cpool.tile([G + 33, N], MM_DT, tag="fB")
            nc.sync.dma_start(out=fA[32:65, :], in_=oha_in[:, :])
            nc.sync.dma_start(out=fB[32:65, : N // 2], in_=ohb_in[:, : N // 2])
            nc.sync.dma_start(out=fB[32:65, N // 2 :], in_=ohb_in[:, N // 2 :])

            # ---- normalize A block ----------------------------------------
            tA3 = tA[:, :].rearrange("p (t d) -> p t d", d=D)
            sqA = wpool.tile([128, RT * D], F32, tag="sqA")
            nc.scalar.activation(sqA[:], tA[:], AF.Square)
            ssA = wpool.tile([128, RT], F32, tag="ssA")
            nc.vector.tensor_reduce(
                ssA[:], sqA[:, :].rearrange("p (t d) -> p t d", d=D), axis=AX.X, op=ALU.add
            )
            nA = wpool.tile([128, RT], F32, tag="nA")
            nc.scalar.activation(nA[:], ssA[:], AF.Sqrt)
            rA = wpool.tile([128, RT], F32, tag="rA")
            nc.vector.reciprocal(rA[:], nA[:])
            an = wpool.tile([128, RT * D], F32, tag="an")
            an3 = an[:, :].rearrange("p (t d) -> p t d", d=D)
            nc.vector.tensor_tensor(
                an3, tA3, rA[:, :].broadcast_to([128, RT, D]), op=ALU.mult
            )

            # ---- normalize B (scaled by -2 for features) -------------------
            tB3 = tB[:, :].rearrange("p (t d) -> p t d", d=D)
            sqB = wpool.tile([128, CT * D], F32, tag="sqB")
            nc.scalar.activation(sqB[:], tB[:], AF.Square)
            ssB = wpool.tile([128, CT], F32, tag="ssB")
            nc.vector.tensor_reduce(
                ssB[:], sqB[:, :].rearrange("p (t d) -> p t d", d=D), axis=AX.X, op=ALU.add
            )
            nB = wpool.tile([128, CT], F32, tag="nB")
            nc.scalar.activation(nB[:], ssB[:], AF.Sqrt)
            rB = wpool.tile([128, CT], F32, tag="rB")
            nc.vector.reciprocal(rB[:], nB[:])
            rBm2 = wpool.tile([128, CT], F32, tag="rBm2")
            nc.vector.tensor_scalar(rBm2[:], rB[:], -2.0, None, op0=ALU.mult)
            bn2 = wpool.tile([128, CT * D], F32, tag="bn2")
            bn23 = bn2[:, :].rearrange("p (t d) -> p t d", d=D)
            nc.vector.tensor_tensor(
                bn23, tB3, rBm2[:, :].broadcast_to([128, CT, D]), op=ALU.mult
            )

            # ---- transpose an -> fA[0:32, :] ------------------------------
            psA = pspool.tile([32, RPC], F32, tag="ps")
            for r in range(RT):
                nc.tensor.transpose(psA[:, r * 128 : (r + 1) * 128], an3[:, r, :], ident[:])
            nc.scalar.copy(fA[0:32, :], psA[:, :])

            # ---- transpose bn2 -> fB[0:32, :] ------------------------------
            for grp in range(CT // 16):
                psB = pspool.tile([32, 16 * 128], F32, tag="ps")
                for k in range(16):
                    t = grp * 16 + k
                    nc.tensor.transpose(
                        psB[:, k * 128 : (k + 1) * 128], bn23[:, t, :], ident[:]
                    )
                nc.scalar.copy(fB[0:32, grp * 2048 : (grp + 1) * 2048], psB[:, :])

            # ---- dist_pos for own rows (first RT tiles of rotated B) ------
            bno = wpool.tile([128, RT * D], F32, tag="bno")
            bno3 = bno[:, :].rearrange("p (t d) -> p t d", d=D)
            nc.vector.tensor_tensor(
                bno3, tB3[:, 0:RT, :], rB[:, 0:RT].broadcast_to([128, RT, D]), op=ALU.mult
            )
            dd = wpool.tile([128, RT * D], F32, tag="dd")
            nc.vector.tensor_tensor(dd[:], an[:], bno[:], op=ALU.subtract)
            sqd = wpool.tile([128, RT * D], F32, tag="sqd")
            nc.scalar.activation(sqd[:], dd[:], AF.Square)
            dp2 = wpool.tile([128, RT], F32, tag="dp2")
            nc.vector.tensor_reduce(
                dp2[:], sqd[:, :].rearrange("p (t d) -> p t d", d=D), axis=AX.X, op=ALU.add
            )
            dpos = wpool.tile([128, RT], F32, tag="dpos")
            nc.scalar.activation(dpos[:], dp2[:], AF.Sqrt)

            # ---- main loop: fused matmul + masked min ----------------------
            mpart = wpool.tile([128, RT * 4], F32, tag="mpart")
            for r in range(RT):
                lhsT = fA[:, r * 128 : (r + 1) * 128]
                for q in range(4):
                    P4 = pspool.tile([128, 2048], F32, tag="ps")
                    for j in range(4):
                        c = q * 4 + j
                        is_diag = q == 0 and j == r // 4
                        nc.tensor.matmul(
                            P4[:, j * 512 : (j + 1) * 512],
                            lhsT,
                            fB[:, c * 512 : (c + 1) * 512],
                            start=True,
                            stop=not is_diag,
                        )
                        if is_diag:
                            off = (r % 4) * 128
                            nc.tensor.matmul(
                                P4[:, j * 512 : (j + 1) * 512],
                                bigi[:],
                                sel[:, 512 - off : 1024 - off],
                                start=False,
                                stop=True,
                            )
                    nc.vector.tensor_reduce(
                        mpart[:, r * 4 + q : r * 4 + q + 1],
                        P4[:, :].rearrange("p (f c) -> p f c", c=512),
                        axis=AX.XY,
                        op=ALU.min,
                    )

            # ---- finalize --------------------------------------------------
            m = wpool.tile([128, RT], F32, tag="m")
            nc.vector.tensor_reduce(
                m[:], mpart[:, :].rearrange("p (r q) -> p r q", q=4), axis=AX.X, op=ALU.min
            )
            mc = wpool.tile([128, RT], F32, tag="mc")
            nc.vector.tensor_scalar(mc[:], m[:], 0.0, None, op0=ALU.max)
            sn = wpool.tile([128, RT], F32, tag="sn")
            nc.scalar.activation(sn[:], mc[:], AF.Sqrt)
            valid = wpool.tile([128, RT], F32, tag="valid")
            nc.vector.tensor_scalar(valid[:], m[:], TH, None, op0=ALU.is_lt)
            dn = wpool.tile([128, RT], F32, tag="dn")
            nc.vector.tensor_tensor(dn[:], sn[:], valid[:], op=ALU.mult)
            pre = wpool.tile([128, RT], F32, tag="pre")
            nc.vector.tensor_tensor(pre[:], dpos[:], dn[:], op=ALU.subtract)
            losses = wpool.tile([128, RT], F32, tag="losses")
            nc.scalar.activation(losses[:], pre[:], AF.Relu, bias=MARGIN)
            nc.sync.dma_start(out=out[:, :], in_=losses[:])

    _split_drain_waits(nc)
    return nc


def _build_nc_sorted(gpc, padg):
    """Group-sorted variant: each core gets `gpc` whole groups, each padded to
    `padg` rows/cols. Only within-group blocks are computed (the masked min
    never needs cross-group pairs). Columns = the core's own rows, so the
    self-pair diagonal sits at block-local positions; it is excluded by an
    in-place +BIG*I add on the 128-wide diagonal slab before the min-reduce.
    Padded columns carry constant-feature 2+BIG -> always excluded.

    Structured as a per-group pipeline: transpose -> feature copy -> matmul ->
    diag add -> min reduce, so PE/ACT/DVE overlap across groups. The B chain
    is emitted first (it gates the feature build); dist_pos is emitted last
    (only needed by the finalize stage)."""
    assert padg <= 512 and padg % 128 == 0
    rmax = gpc * padg          # rows (and cols) per core
    rt = rmax // 128           # 128-row tiles per core
    tpg = padg // 128          # row tiles per group

    nc = bass.Bass()
    a_in = nc.dram_tensor("a", [128, rt * D], F32, kind="ExternalInput")
    b_in = nc.dram_tensor("b", [128, rt * D], F32, kind="ExternalInput")
    cv_in = nc.dram_tensor("cv", [2, rmax], MM_DT, kind="ExternalInput")
    out = nc.dram_tensor("losses", [128, rt], F32, kind="ExternalOutput")

    ident_np = np.eye(128, dtype=np.float32)
    seld_np = (BIG * np.eye(128)).astype(np.float32)
    ident_d = nc.inline_tensor(ident_np, name="identc")
    seld_d = nc.inline_tensor(seld_np, name="seldc")

    half = rt * D // 2

    with TileContext(nc) as tc:
        with (
            tc.tile_pool(name="const", bufs=1) as cpool,
            tc.tile_pool(name="work", bufs=1) as wpool,
            tc.tile_pool(name="pst", bufs=2, space="PSUM") as pstp,
            tc.tile_pool(name="psm", bufs=4, space="PSUM") as psmp,
        ):
            ident = cpool.tile([128, 128], F32, tag="ident")
            nc.sync.dma_start(out=ident[:], in_=ident_d[:, :])
            seld = cpool.tile([128, 128], F32, tag="seld")
            nc.sync.dma_start(out=seld[:], in_=seld_d[:, :])

            fA = cpool.tile([33, rmax], MM_DT, tag="fA")
            fB = cpool.tile([33, rmax], MM_DT, tag="fB")
            nc.sync.dma_start(out=fB[32:33, :], in_=cv_in[1:2, :])
            nc.sync.dma_start(out=fA[32:33, :], in_=cv_in[0:1, :])

            # ---- B chain (critical: gates the feature build) ----
            tB = wpool.tile([128, rt * D], F32, tag="tB")
            nc.sync.dma_start(out=tB[:, :half], in_=b_in[:, :half])
            nc.sync.dma_start(out=tB[:, half:], in_=b_in[:, half:])
            tB3 = tB[:, :].rearrange("p (t d) -> p t d", d=D)
            sqB = wpool.tile([128, rt * D], F32, tag="sqB")
            nc.scalar.activation(sqB[:, :half], tB[:, :half], AF.Square)
            nc.scalar.activation(sqB[:, half:], tB[:, half:], AF.Square)
            ssB = wpool.tile([128, rt], F32, tag="ssB")
            nc.vector.tensor_reduce(
                ssB[:], sqB[:, :].rearrange("p (t d) -> p t d", d=D), axis=AX.X, op=ALU.add
            )
            nB = wpool.tile([128, rt], F32, tag="nB")
            nc.scalar.activation(nB[:], ssB[:], AF.Sqrt)
            rB = wpool.tile([128, rt], F32, tag="rB")
            nc.vector.reciprocal(rB[:], nB[:])
            rBm2 = wpool.tile([128, rt], F32, tag="rBm2")
            nc.vector.tensor_scalar(rBm2[:], rB[:], -2.0, None, op0=ALU.mult)
            bn2 = wpool.tile([128, rt * D], F32, tag="bn2")
            bn23 = bn2[:, :].rearrange("p (t d) -> p t d", d=D)
            nc.vector.tensor_tensor(
                bn23, tB3, rBm2[:, :].broadcast_to([128, rt, D]), op=ALU.mult
            )

            # ---- A chain ----
            tA = wpool.tile([128, rt * D], F32, tag="tA")
            nc.sync.dma_start(out=tA[:, :half], in_=a_in[:, :half])
            nc.sync.dma_start(out=tA[:, half:], in_=a_in[:, half:])
            tA3 = tA[:, :].rearrange("p (t d) -> p t d", d=D)
            sqA = wpool.tile([128, rt * D], F32, tag="sqA")
            nc.scalar.activation(sqA[:, :half], tA[:, :half], AF.Square)
            nc.scalar.activation(sqA[:, half:], tA[:, half:], AF.Square)
            ssA = wpool.tile([128, rt], F32, tag="ssA")
            nc.vector.tensor_reduce(
                ssA[:], sqA[:, :].rearrange("p (t d) -> p t d", d=D), axis=AX.X, op=ALU.add
            )
            nA = wpool.tile([128, rt], F32, tag="nA")
            nc.scalar.activation(nA[:], ssA[:], AF.Sqrt)
            rA = wpool.tile([128, rt], F32, tag="rA")
            nc.vector.reciprocal(rA[:], nA[:])
            an = wpool.tile([128, rt * D], F32, tag="an")
            an3 = an[:, :].rearrange("p (t d) -> p t d", d=D)
            nc.vector.tensor_tensor(
                an3, tA3, rA[:, :].broadcast_to([128, rt, D]), op=ALU.mult
            )

            # ---- per-group pipeline ----
            mpart = wpool.tile([128, rt], F32, tag="mpart")
            for gl in range(gpc):
                base = gl * tpg
                cs = gl * padg
                psB = pstp.tile([32, padg], F32, tag="pstB")
                for r in range(tpg):
                    nc.tensor.transpose(
                        psB[:, r * 128 : (r + 1) * 128], bn23[:, base + r, :], ident[:]
                    )
                nc.scalar.copy(fB[0:32, cs : cs + padg], psB[:, :])
                psA = pstp.tile([32, padg], F32, tag="pstA")
                for r in range(tpg):
                    nc.tensor.transpose(
                        psA[:, r * 128 : (r + 1) * 128], an3[:, base + r, :], ident[:]
                    )
                nc.scalar.copy(fA[0:32, cs : cs + padg], psA[:, :])
                for r in range(tpg):
                    idx = base + r
                    off = r * 128
                    P = psmp.tile([128, 512], F32, tag="psm")
                    nc.tensor.matmul(
                        P[:, :padg],
                        fA[:, idx * 128 : (idx + 1) * 128],
                        fB[:, cs : cs + padg],
                        start=True,
                        stop=True,
                    )
                    nc.vector.tensor_tensor(
                        P[:, off : off + 128], P[:, off : off + 128], seld[:], op=ALU.add
                    )
                    nc.vector.tensor_reduce(
                        mpart[:, idx : idx + 1], P[:, :padg], axis=AX.X, op=ALU.min
                    )

            # ---- dist_pos (off critical path): || an - bn || ----
            bno = wpool.tile([128, rt * D], F32, tag="bno")
            nc.vector.tensor_tensor(
                bno[:, :].rearrange("p (t d) -> p t d", d=D),
                tB3,
                rB[:, :].broadcast_to([128, rt, D]),
                op=ALU.mult,
            )
            dd = wpool.tile([128, rt * D], F32, tag="dd")
            nc.vector.tensor_tensor(dd[:], an[:], bno[:], op=ALU.subtract)
            sqd = wpool.tile([128, rt * D], F32, tag="sqd")
            nc.scalar.activation(sqd[:], dd[:], AF.Square)
            dp2 = wpool.tile([128, rt], F32, tag="dp2")
            nc.vector.tensor_reduce(
                dp2[:], sqd[:, :].rearrange("p (t d) -> p t d", d=D), axis=AX.X, op=ALU.add
            )
            dpos = wpool.tile([128, rt], F32, tag="dpos")
            nc.scalar.activation(dpos[:], dp2[:], AF.Sqrt)

            # ---- finalize ----
            mc = wpool.tile([128, rt], F32, tag="mc")
            nc.vector.tensor_scalar(mc[:], mpart[:], 0.0, None, op0=ALU.max)
            sn = wpool.tile([128, rt], F32, tag="sn")
            nc.scalar.activation(sn[:], mc[:], AF.Sqrt)
            valid = wpool.tile([128, rt], F32, tag="valid")
            nc.vector.tensor_scalar(valid[:], mpart[:], TH, None, op0=ALU.is_lt)
            dn = wpool.tile([128, rt], F32, tag="dn")
            nc.vector.tensor_tensor(dn[:], sn[:], valid[:], op=ALU.mult)
            pre = wpool.tile([128, rt], F32, tag="pre")
            nc.vector.tensor_tensor(pre[:], dpos[:], dn[:], op=ALU.subtract)
            losses = wpool.tile([128, rt], F32, tag="losses")
            nc.scalar.activation(losses[:], pre[:], AF.Relu, bias=MARGIN)
            nc.sync.dma_start(out=out[:, :], in_=losses[:])

    _split_drain_waits(nc)
    return nc


_NC_CACHE = None
_NC_SORTED_CACHE = {}


def _get_nc():
    global _NC_CACHE
    if _NC_CACHE is None:
        _NC_CACHE = _build_nc()
    return _NC_CACHE


def _get_nc_sorted(gpc, padg):
    key = (gpc, padg)
    if key not in _NC_SORTED_CACHE:
        _NC_SORTED_CACHE[key] = _build_nc_sorted(gpc, padg)
    return _NC_SORTED_CACHE[key]


def _tile_tp(x):
    """[R, 32] rows -> [128, (R/128)*32] with row t*128+p on partition p."""
    r = x.shape[0]
    return (
        np.ascontiguousarray(
            x.reshape(r // 128, 128, D).transpose(1, 0, 2).reshape(128, (r // 128) * D)
        )
    )


def _kernel_sorted(A, B, lab):
    counts = np.bincount(lab, minlength=G)
    gn = len(counts)
    gpc = -(-gn // NCORES)
    padg = max(128, -(-int(counts.max()) // 128) * 128)
    if padg > 512:
        return None  # degenerate label distribution: fall back to full kernel
    rmax = gpc * padg
    rt = rmax // 128

    order = np.argsort(lab, kind="stable")
    starts = np.concatenate([[0], np.cumsum(counts)])

    src = np.full((NCORES, rmax), -1, np.int64)
    for g in range(gn):
        c, gl = divmod(g, gpc)
        n = int(counts[g])
        src[c, gl * padg : gl * padg + n] = order[starts[g] : starts[g] + n]

    in_maps = []
    for c in range(NCORES):
        idx = src[c]
        real = idx >= 0
        a_rows = np.ones((rmax, D), np.float32)
        b_rows = np.ones((rmax, D), np.float32)
        a_rows[real] = A[idx[real]]
        b_rows[real] = B[idx[real]]
        cv = np.ones((2, rmax), np.float32)
        cv[1] = np.where(real, 2.0, 2.0 + BIG)
        in_maps.append(
            {
                "a": _tile_tp(a_rows),
                "b": _tile_tp(b_rows),
                "cv": np.ascontiguousarray(cv),
            }
        )

    global _last_in_maps, _last_nc
    _last_in_maps = in_maps
    nc = _get_nc_sorted(gpc, padg)
    _last_nc = nc
    res = run_bass_kernel_spmd(nc, in_maps, list(range(NCORES)))
    total = 0.0
    for c in range(NCORES):
        lo = res.results[c]["losses"]  # [128, rt]; [p, t] = loss of local row t*128+p
        flat = lo.T.reshape(rmax)
        real = src[c] >= 0
        total += float(flat[real].sum(dtype=np.float64))
    return np.float32(total / N)


def kernel(A=None, B=None, labels=None, **_unused):
    import os

    A = np.asarray(A, dtype=np.float32)
    B = np.asarray(B, dtype=np.float32)
    lab = np.asarray(labels).astype(np.int64)
    lab = lab - lab.min() if lab.min() < 0 else lab

    if not os.environ.get("KERNEL_FORCE_FULL"):
        out = _kernel_sorted(A, B, lab.astype(np.int32))
        if out is not None:
            return out

    eye = np.arange(G, dtype=np.int32)
    in_maps = []
    for c in range(NCORES):
        rows = slice(c * RPC, (c + 1) * RPC)
        a_c = _tile_tp(A[rows])
        b_rot = np.roll(B, -c * RPC, axis=0)
        lab_rot = np.roll(lab, -c * RPC)
        b_c = _tile_tp(b_rot)
        oha = np.concatenate(
            [
                np.ones((1, RPC), np.float32),
                (-BIG) * (lab[rows][None, :] == eye[:, None]).astype(np.float32),
            ]
        )
        ohb = np.concatenate(
            [
                np.full((1, N), 2.0 + BIG, np.float32),
                (lab_rot[None, :] == eye[:, None]).astype(np.float32),
            ]
        )
        in_maps.append(
            {
                "a": a_c,
                "b": b_c,
                "oha": np.ascontiguousarray(oha),
                "ohb": np.ascontiguousarray(ohb),
            }
        )

    global _last_in_maps, _last_nc
    _last_in_maps = in_maps
    nc = _get_nc()
    _last_nc = nc
    res = run_bass_kernel_spmd(nc, in_maps, list(range(NCORES)))
    total = 0.0
    for c in range(NCORES):
        lo = res.results[c]["losses"]  # [128, RT]; [p, r] = loss of row r*128+p
        total += float(lo.sum(dtype=np.float64))
    return np.float32(total / N)


# revision 22
# speedup vs baseline: 5.0850x; 1.1167x over previous
"""Grouped triplet loss on 8 trn2 NeuronCores.

Strategy (data-parallel over A rows, hint-compliant):
  - Each core takes a 1024-row block of A, full B (column-rotated so the
    diagonal of the distance matrix lands at core-independent positions).
  - L2 normalization of A-block and B on device.
  - One fused matmul per (row-tile, col-chunk) computes the *masked* squared
    distance directly in PSUM via extended feature vectors:
        F_A = [ a_i (32) | 1 | -BIG*onehot(label_i) (32) ]   (K = 65)
        F_B = [ -2*b_j   | 2+BIG |      onehot(label_j)  ]
    so PSUM = 2 - 2*a.b + BIG*(1 - same_group).
  - A tiny bf16 identity matmul accumulates +BIG on the diagonal (self-pair
    exclusion).
  - DVE min-reduces PSUM (4 banks per op); rows with min >= TH had no valid
    negative -> dist_neg = 0 (matches torch "skip groups of size < 2").
  - losses = relu(dist_pos - dist_neg + margin); host averages.

Host-side work is limited to sharding/layout: slicing, row-rotation, (t p)
tiling, and one-hot encoding of the integer labels. All float math happens
on device.
"""

import numpy as np

import concourse.bass as bass
import concourse.mybir as mybir
from concourse.tile import TileContext
from concourse.bass_utils import run_bass_kernel_spmd

N, D, G = 8192, 32, 32
NCORES = 8
RPC = N // NCORES      # rows per core = 1024
RT = RPC // 128        # row tiles per core = 8
CT = N // 128          # column tiles = 64
NCHUNK = N // 512      # matmul column chunks = 16
BIG = 64.0
TH = 32.0
MARGIN = 1.0

F32 = mybir.dt.float32
BF16 = mybir.dt.bfloat16
AF = mybir.ActivationFunctionType
ALU = mybir.AluOpType
AX = mybir.AxisListType

MM_DT = mybir.dt.float32r  # matmul feature dtype (float32 | float32r)

_MAX_DRAIN_WAITS = 1


def _split_drain_waits(nc):
    """This container's walrus rejects any instruction with >1 sem-wait.
    Hoist excess waits onto preceding same-engine single-wait Drains."""
    nsplit = 0
    for f in nc.m.functions:
        for bb in f.blocks:
            new_insts = []
            for inst in bb.instructions:
                si = inst.sync_info
                waits = list(si.on_wait) if si and si.on_wait else []
                if len(waits) > _MAX_DRAIN_WAITS:
                    extra, keep = waits[:-_MAX_DRAIN_WAITS], waits[-_MAX_DRAIN_WAITS:]
                    for w in extra:
                        d = mybir.InstDrain(
                            name=f"{inst.name}-swsplit{nsplit}",
                            engine=inst.engine,
                            ins=[],
                            outs=[],
                            sync_info=mybir.SyncInfo(on_wait=[w], on_update=[]),
                        )
                        nsplit += 1
                        nc.register_instruction(d, overwrite=True)
                        new_insts.append(d)
                    si.on_wait = keep
                new_insts.append(inst)
            bb.instructions[:] = new_insts


def _build_nc():
    import ml_dtypes

    nc = bass.Bass()

    a_in = nc.dram_tensor("a", [128, RT * D], F32, kind="ExternalInput")
    b_in = nc.dram_tensor("b", [128, CT * D], F32, kind="ExternalInput")
    # row 0: constant feature (1 for A, 2+BIG for B); rows 1..32: one-hot
    oha_in = nc.dram_tensor("oha", [G + 1, RPC], MM_DT, kind="ExternalInput")
    ohb_in = nc.dram_tensor("ohb", [G + 1, N], MM_DT, kind="ExternalInput")
    out = nc.dram_tensor("losses", [128, RT], F32, kind="ExternalOutput")

    ident_np = np.eye(128, dtype=np.float32)
    sel_np = np.zeros((128, 1024), dtype=np.float32)
    sel_np[np.arange(128), 512 + np.arange(128)] = 1.0
    bigi_np = (BIG * np.eye(128)).astype(ml_dtypes.bfloat16)
    ident_d = nc.inline_tensor(ident_np, name="identc")
    sel_d = nc.inline_tensor(sel_np.astype(ml_dtypes.bfloat16), name="selc")
    bigi_d = nc.inline_tensor(bigi_np, name="bigic")

    with TileContext(nc) as tc:
        with (
            tc.tile_pool(name="const", bufs=1) as cpool,
            tc.tile_pool(name="work", bufs=1) as wpool,
            tc.tile_pool(name="ps", bufs=2, space="PSUM") as pspool,
        ):
            # ---- constants -------------------------------------------------
            ident = cpool.tile([128, 128], F32, tag="ident")
            nc.sync.dma_start(out=ident[:], in_=ident_d[:, :])
            sel = cpool.tile([128, 1024], BF16, tag="sel")
            nc.sync.dma_start(out=sel[:], in_=sel_d[:, :])
            bigi = cpool.tile([128, 128], BF16, tag="bigi")
            nc.sync.dma_start(out=bigi[:], in_=bigi_d[:, :])

            # ---- raw loads -------------------------------------------------
            tA = wpool.tile([128, RT * D], F32, tag="tA")
            nc.sync.dma_start(out=tA[:], in_=a_in[:, :])
            tB = wpool.tile([128, CT * D], F32, tag="tB")
            # split into 2 DMAs to use more queues
            nc.sync.dma_start(out=tB[:, : CT * D // 2], in_=b_in[:, : CT * D // 2])
            nc.sync.dma_start(out=tB[:, CT * D // 2 :], in_=b_in[:, CT * D // 2 :])

            fA = cpool.tile([G + 33, RPC], MM_DT, tag="fA")
            fB = cpool.tile([G + 33, N], MM_DT, tag="fB")
            nc.sync.dma_start(out=fA[32:65, :], in_=oha_in[:, :])
            nc.sync.dma_start(out=fB[32:65, : N // 2], in_=ohb_in[:, : N // 2])
            nc.sync.dma_start(out=fB[32:65, N // 2 :], in_=ohb_in[:, N // 2 :])

            # ---- normalize A block ----------------------------------------
            tA3 = tA[:, :].rearrange("p (t d) -> p t d", d=D)
            sqA = wpool.tile([128, RT * D], F32, tag="sqA")
            nc.scalar.activation(sqA[:], tA[:], AF.Square)
            ssA = wpool.tile([128, RT], F32, tag="ssA")
            nc.vector.tensor_reduce(
                ssA[:], sqA[:, :].rearrange("p (t d) -> p t d", d=D), axis=AX.X, op=ALU.add
            )
            nA = wpool.tile([128, RT], F32, tag="nA")
            nc.scalar.activation(nA[:], ssA[:], AF.Sqrt)
            rA = wpool.tile([128, RT], F32, tag="rA")
            nc.vector.reciprocal(rA[:], nA[:])
            an = wpool.tile([128, RT * D], F32, tag="an")
            an3 = an[:, :].rearrange("p (t d) -> p t d", d=D)
            nc.vector.tensor_tensor(
                an3, tA3, rA[:, :].broadcast_to([128, RT, D]), op=ALU.mult
            )

            # ---- normalize B (scaled by -2 for features) -------------------
            tB3 = tB[:, :].rearrange("p (t d) -> p t d", d=D)
            sqB = wpool.tile([128, CT * D], F32, tag="sqB")
            nc.scalar.activation(sqB[:], tB[:], AF.Square)
            ssB = wpool.tile([128, CT], F32, tag="ssB")
            nc.vector.tensor_reduce(
                ssB[:], sqB[:, :].rearrange("p (t d) -> p t d", d=D), axis=AX.X, op=ALU.add
            )
            nB = wpool.tile([128, CT], F32, tag="nB")
            nc.scalar.activation(nB[:], ssB[:], AF.Sqrt)
            rB = wpool.tile([128, CT], F32, tag="rB")
            nc.vector.reciprocal(rB[:], nB[:])
            rBm2 = wpool.tile([128, CT], F32, tag="rBm2")
            nc.vector.tensor_scalar(rBm2[:], rB[:], -2.0, None, op0=ALU.mult)
            bn2 = wpool.tile([128, CT * D], F32, tag="bn2")
            bn23 = bn2[:, :].rearrange("p (t d) -> p t d", d=D)
            nc.vector.tensor_tensor(
                bn23, tB3, rBm2[:, :].broadcast_to([128, CT, D]), op=ALU.mult
            )

            # ---- transpose an -> fA[0:32, :] ------------------------------
            psA = pspool.tile([32, RPC], F32, tag="ps")
            for r in range(RT):
                nc.tensor.transpose(psA[:, r * 128 : (r + 1) * 128], an3[:, r, :], ident[:])
            nc.scalar.copy(fA[0:32, :], psA[:, :])

            # ---- transpose bn2 -> fB[0:32, :] ------------------------------
            for grp in range(CT // 16):
                psB = pspool.tile([32, 16 * 128], F32, tag="ps")
                for k in range(16):
                    t = grp * 16 + k
                    nc.tensor.transpose(
                        psB[:, k * 128 : (k + 1) * 128], bn23[:, t, :], ident[:]
                    )
                nc.scalar.copy(fB[0:32, grp * 2048 : (grp + 1) * 2048], psB[:, :])

            # ---- dist_pos for own rows (first RT tiles of rotated B) ------
            bno = wpool.tile([128, RT * D], F32, tag="bno")
            bno3 = bno[:, :].rearrange("p (t d) -> p t d", d=D)
            nc.vector.tensor_tensor(
                bno3, tB3[:, 0:RT, :], rB[:, 0:RT].broadcast_to([128, RT, D]), op=ALU.mult
            )
            dd = wpool.tile([128, RT * D], F32, tag="dd")
            nc.vector.tensor_tensor(dd[:], an[:], bno[:], op=ALU.subtract)
            sqd = wpool.tile([128, RT * D], F32, tag="sqd")
            nc.scalar.activation(sqd[:], dd[:], AF.Square)
            dp2 = wpool.tile([128, RT], F32, tag="dp2")
            nc.vector.tensor_reduce(
                dp2[:], sqd[:, :].rearrange("p (t d) -> p t d", d=D), axis=AX.X, op=ALU.add
            )
            dpos = wpool.tile([128, RT], F32, tag="dpos")
            nc.scalar.activation(dpos[:], dp2[:], AF.Sqrt)

            # ---- main loop: fused matmul + masked min ----------------------
            mpart = wpool.tile([128, RT * 4], F32, tag="mpart")
            for r in range(RT):
                lhsT = fA[:, r * 128 : (r + 1) * 128]
                for q in range(4):
                    P4 = pspool.tile([128, 2048], F32, tag="ps")
                    for j in range(4):
                        c = q * 4 + j
                        is_diag = q == 0 and j == r // 4
                        nc.tensor.matmul(
                            P4[:, j * 512 : (j + 1) * 512],
                            lhsT,
                            fB[:, c * 512 : (c + 1) * 512],
                            start=True,
                            stop=not is_diag,
                        )
                        if is_diag:
                            off = (r % 4) * 128
                            nc.tensor.matmul(
                                P4[:, j * 512 : (j + 1) * 512],
                                bigi[:],
                                sel[:, 512 - off : 1024 - off],
                                start=False,
                                stop=True,
                            )
                    nc.vector.tensor_reduce(
                        mpart[:, r * 4 + q : r * 4 + q + 1],
                        P4[:, :].rearrange("p (f c) -> p f c", c=512),
                        axis=AX.XY,
                        op=ALU.min,
                    )

            # ---- finalize --------------------------------------------------
            m = wpool.tile([128, RT], F32, tag="m")
            nc.vector.tensor_reduce(
                m[:], mpart[:, :].rearrange("p (r q) -> p r q", q=4), axis=AX.X, op=ALU.min
            )
            mc = wpool.tile([128, RT], F32, tag="mc")
            nc.vector.tensor_scalar(mc[:], m[:], 0.0, None, op0=ALU.max)
            sn = wpool.tile([128, RT], F32, tag="sn")
            nc.scalar.activation(sn[:], mc[:], AF.Sqrt)
            valid = wpool.tile([128, RT], F32, tag="valid")
            nc.vector.tensor_scalar(valid[:], m[:], TH, None, op0=ALU.is_lt)
            dn = wpool.tile([128, RT], F32, tag="dn")
            nc.vector.tensor_tensor(dn[:], sn[:], valid[:], op=ALU.mult)
            pre = wpool.tile([128, RT], F32, tag="pre")
            nc.vector.tensor_tensor(pre[:], dpos[:], dn[:], op=ALU.subtract)
            losses = wpool.tile([128, RT], F32, tag="losses")
            nc.scalar.activation(losses[:], pre[:], AF.Relu, bias=MARGIN)
            nc.sync.dma_start(out=out[:, :], in_=losses[:])

    _split_drain_waits(nc)
    return nc


def _build_nc_sorted(gpc, padg):
    """Group-sorted variant: each core gets `gpc` whole groups, each padded to
    `padg` rows/cols. Only within-group blocks are computed (the masked min
    never needs cross-group pairs). Columns = the core's own rows, so the
    self-pair diagonal sits at block-local positions; it is excluded by an
    in-place +BIG*I add on the 128-wide diagonal slab before the min-reduce.
    Padded columns carry constant-feature 2+BIG -> always excluded.

    Structured as a per-group pipeline: transpose -> feature copy -> matmul ->
    diag add -> min reduce, so PE/ACT/DVE overlap across groups. The B chain
    is emitted first (it gates the feature build); dist_pos is emitted last
    (only needed by the finalize stage)."""
    assert padg <= 512 and padg % 128 == 0
    rmax = gpc * padg          # rows (and cols) per core
    rt = rmax // 128           # 128-row tiles per core
    tpg = padg // 128          # row tiles per group

    nc = bass.Bass()
    a_in = nc.dram_tensor("a", [128, rt * D], F32, kind="ExternalInput")
    b_in = nc.dram_tensor("b", [128, rt * D], F32, kind="ExternalInput")
    cv_in = nc.dram_tensor("cv", [2, rmax], MM_DT, kind="ExternalInput")
    out = nc.dram_tensor("losses", [128, rt], F32, kind="ExternalOutput")

    ident_np = np.eye(128, dtype=np.float32)
    seld_np = (BIG * np.eye(128)).astype(np.float32)
    ident_d = nc.inline_tensor(ident_np, name="identc")
    seld_d = nc.inline_tensor(seld_np, name="seldc")

    half = rt * D // 2

    with TileContext(nc) as tc:
        with (
            tc.tile_pool(name="const", bufs=1) as cpool,
            tc.tile_pool(name="work", bufs=1) as wpool,
            tc.tile_pool(name="pst", bufs=2, space="PSUM") as pstp,
            tc.tile_pool(name="psm", bufs=4, space="PSUM") as psmp,
        ):
            # input DMAs first, spread across otherwise-idle engine queues
            tB = wpool.tile([128, rt * D], F32, tag="tB")
            nc.sync.dma_start(out=tB[:, :half], in_=b_in[:, :half])
            nc.sync.dma_start(out=tB[:, half:], in_=b_in[:, half:])
            tA = wpool.tile([128, rt * D], F32, tag="tA")
            nc.gpsimd.dma_start(out=tA[:, :half], in_=a_in[:, :half])
            nc.gpsimd.dma_start(out=tA[:, half:], in_=a_in[:, half:])

            ident = cpool.tile([128, 128], F32, tag="ident")
            nc.scalar.dma_start(out=ident[:], in_=ident_d[:, :])
            seld = cpool.tile([128, 128], F32, tag="seld")
            nc.scalar.dma_start(out=seld[:], in_=seld_d[:, :])

            fA = cpool.tile([33, rmax], MM_DT, tag="fA")
            fB = cpool.tile([33, rmax], MM_DT, tag="fB")
            nc.scalar.dma_start(out=fB[32:33, :], in_=cv_in[1:2, :])
            nc.scalar.dma_start(out=fA[32:33, :], in_=cv_in[0:1, :])

            # fire the ACT table load immediately (contents irrelevant)
            warmup_act = wpool.tile([128, 8], F32, tag="warmup_act")
            nc.scalar.activation(warmup_act[:], warmup_act[:], AF.Square)

            # ---- B chain (critical: gates the feature build) ----
            tB3 = tB[:, :].rearrange("p (t d) -> p t d", d=D)
            sqB = wpool.tile([128, rt * D], F32, tag="sqB")
            nc.scalar.activation(sqB[:, :half], tB[:, :half], AF.Square)
            nc.scalar.activation(sqB[:, half:], tB[:, half:], AF.Square)
            ssB = wpool.tile([128, rt], F32, tag="ssB")
            nc.vector.tensor_reduce(
                ssB[:], sqB[:, :].rearrange("p (t d) -> p t d", d=D), axis=AX.X, op=ALU.add
            )
            nB = wpool.tile([128, rt], F32, tag="nB")
            nc.scalar.activation(nB[:], ssB[:], AF.Sqrt)
            rB = wpool.tile([128, rt], F32, tag="rB")
            nc.vector.reciprocal(rB[:], nB[:])
            rBm2 = wpool.tile([128, rt], F32, tag="rBm2")
            nc.vector.tensor_scalar(rBm2[:], rB[:], -2.0, None, op0=ALU.mult)
            bn2 = wpool.tile([128, rt * D], F32, tag="bn2")
            bn23 = bn2[:, :].rearrange("p (t d) -> p t d", d=D)
            nc.vector.tensor_tensor(
                bn23, tB3, rBm2[:, :].broadcast_to([128, rt, D]), op=ALU.mult
            )

            # ---- A chain ----
            tA3 = tA[:, :].rearrange("p (t d) -> p t d", d=D)
            sqA = wpool.tile([128, rt * D], F32, tag="sqA")
            nc.scalar.activation(sqA[:, :half], tA[:, :half], AF.Square)
            nc.scalar.activation(sqA[:, half:], tA[:, half:], AF.Square)
            ssA = wpool.tile([128, rt], F32, tag="ssA")
            nc.vector.tensor_reduce(
                ssA[:], sqA[:, :].rearrange("p (t d) -> p t d", d=D), axis=AX.X, op=ALU.add
            )
            nA = wpool.tile([128, rt], F32, tag="nA")
            nc.scalar.activation(nA[:], ssA[:], AF.Sqrt)
            rA = wpool.tile([128, rt], F32, tag="rA")
            nc.vector.reciprocal(rA[:], nA[:])
            an = wpool.tile([128, rt * D], F32, tag="an")
            an3 = an[:, :].rearrange("p (t d) -> p t d", d=D)
            nc.vector.tensor_tensor(
                an3, tA3, rA[:, :].broadcast_to([128, rt, D]), op=ALU.mult
            )

            # ---- PE warm-up: dummy transposes keyed to sqB so the HAM
            # clock-gate opens before the real transposes/matmuls arrive ----
            for w in range(16):
                pw = psmp.tile([128, 512], F32, tag="psm")
                nc.tensor.transpose(pw[:, 0:128], sqB[:, 0:128], ident[:])

            # ---- per-group pipeline ----
            mpart = wpool.tile([128, rt], F32, tag="mpart")
            for gl in range(gpc):
                base = gl * tpg
                cs = gl * padg
                psB = pstp.tile([32, padg], F32, tag="pstB")
                for r in range(tpg):
                    nc.tensor.transpose(
                        psB[:, r * 128 : (r + 1) * 128], bn23[:, base + r, :], ident[:]
                    )
                nc.scalar.copy(fB[0:32, cs : cs + padg], psB[:, :])
                psA = pstp.tile([32, padg], F32, tag="pstA")
                for r in range(tpg):
                    nc.tensor.transpose(
                        psA[:, r * 128 : (r + 1) * 128], an3[:, base + r, :], ident[:]
                    )
                nc.scalar.copy(fA[0:32, cs : cs + padg], psA[:, :])
                for r in range(tpg):
                    idx = base + r
                    off = r * 128
                    P = psmp.tile([128, 512], F32, tag="psm")
                    nc.tensor.matmul(
                        P[:, :padg],
                        fA[:, idx * 128 : (idx + 1) * 128],
                        fB[:, cs : cs + padg],
                        start=True,
                        stop=True,
                    )
                    nc.vector.tensor_tensor(
                        P[:, off : off + 128], P[:, off : off + 128], seld[:], op=ALU.add
                    )
                    nc.vector.tensor_reduce(
                        mpart[:, idx : idx + 1], P[:, :padg], axis=AX.X, op=ALU.min
                    )

            # ---- dist_pos (off critical path): || an - bn || ----
            bno = wpool.tile([128, rt * D], F32, tag="bno")
            nc.vector.tensor_tensor(
                bno[:, :].rearrange("p (t d) -> p t d", d=D),
                tB3,
                rB[:, :].broadcast_to([128, rt, D]),
                op=ALU.mult,
            )
            dd = wpool.tile([128, rt * D], F32, tag="dd")
            nc.vector.tensor_tensor(dd[:], an[:], bno[:], op=ALU.subtract)
            sqd = wpool.tile([128, rt * D], F32, tag="sqd")
            nc.scalar.activation(sqd[:], dd[:], AF.Square)
            dp2 = wpool.tile([128, rt], F32, tag="dp2")
            nc.vector.tensor_reduce(
                dp2[:], sqd[:, :].rearrange("p (t d) -> p t d", d=D), axis=AX.X, op=ALU.add
            )
            dpos = wpool.tile([128, rt], F32, tag="dpos")
            nc.scalar.activation(dpos[:], dp2[:], AF.Sqrt)

            # ---- finalize ----
            mc = wpool.tile([128, rt], F32, tag="mc")
            nc.vector.tensor_scalar(mc[:], mpart[:], 0.0, None, op0=ALU.max)
            sn = wpool.tile([128, rt], F32, tag="sn")
            nc.scalar.activation(sn[:], mc[:], AF.Sqrt)
            valid = wpool.tile([128, rt], F32, tag="valid")
            nc.vector.tensor_scalar(valid[:], mpart[:], TH, None, op0=ALU.is_lt)
            dn = wpool.tile([128, rt], F32, tag="dn")
            nc.vector.tensor_tensor(dn[:], sn[:], valid[:], op=ALU.mult)
            pre = wpool.tile([128, rt], F32, tag="pre")
            nc.vector.tensor_tensor(pre[:], dpos[:], dn[:], op=ALU.subtract)
            losses = wpool.tile([128, rt], F32, tag="losses")
            nc.scalar.activation(losses[:], pre[:], AF.Relu, bias=MARGIN)
            nc.sync.dma_start(out=out[:, :], in_=losses[:])

    _split_drain_waits(nc)
    return nc


_NC_CACHE = None
_NC_SORTED_CACHE = {}


def _get_nc():
    global _NC_CACHE
    if _NC_CACHE is None:
        _NC_CACHE = _build_nc()
    return _NC_CACHE


def _get_nc_sorted(gpc, padg):
    key = (gpc, padg)
    if key not in _NC_SORTED_CACHE:
        _NC_SORTED_CACHE[key] = _build_nc_sorted(gpc, padg)
    return _NC_SORTED_CACHE[key]


def _tile_tp(x):
    """[R, 32] rows -> [128, (R/128)*32] with row t*128+p on partition p."""
    r = x.shape[0]
    return (
        np.ascontiguousarray(
            x.reshape(r // 128, 128, D).transpose(1, 0, 2).reshape(128, (r // 128) * D)
        )
    )


def _kernel_sorted(A, B, lab):
    counts = np.bincount(lab, minlength=G)
    gn = len(counts)
    gpc = -(-gn // NCORES)
    padg = max(128, -(-int(counts.max()) // 128) * 128)
    if padg > 512:
        return None  # degenerate label distribution: fall back to full kernel
    rmax = gpc * padg
    rt = rmax // 128

    order = np.argsort(lab, kind="stable")
    starts = np.concatenate([[0], np.cumsum(counts)])

    src = np.full((NCORES, rmax), -1, np.int64)
    for g in range(gn):
        c, gl = divmod(g, gpc)
        n = int(counts[g])
        src[c, gl * padg : gl * padg + n] = order[starts[g] : starts[g] + n]

    in_maps = []
    for c in range(NCORES):
        idx = src[c]
        real = idx >= 0
        a_rows = np.ones((rmax, D), np.float32)
        b_rows = np.ones((rmax, D), np.float32)
        a_rows[real] = A[idx[real]]
        b_rows[real] = B[idx[real]]
        cv = np.ones((2, rmax), np.float32)
        cv[1] = np.where(real, 2.0, 2.0 + BIG)
        in_maps.append(
            {
                "a": _tile_tp(a_rows),
                "b": _tile_tp(b_rows),
                "cv": np.ascontiguousarray(cv),
            }
        )

    global _last_in_maps, _last_nc
    _last_in_maps = in_maps
    nc = _get_nc_sorted(gpc, padg)
    _last_nc = nc
    res = run_bass_kernel_spmd(nc, in_maps, list(range(NCORES)))
    total = 0.0
    for c in range(NCORES):
        lo = res.results[c]["losses"]  # [128, rt]; [p, t] = loss of local row t*128+p
        flat = lo.T.reshape(rmax)
        real = src[c] >= 0
        total += float(flat[real].sum(dtype=np.float64))
    return np.float32(total / N)


def kernel(A=None, B=None, labels=None, **_unused):
    import os

    A = np.asarray(A, dtype=np.float32)
    B = np.asarray(B, dtype=np.float32)
    lab = np.asarray(labels).astype(np.int64)
    lab = lab - lab.min() if lab.min() < 0 else lab

    if not os.environ.get("KERNEL_FORCE_FULL"):
        out = _kernel_sorted(A, B, lab.astype(np.int32))
        if out is not None:
            return out

    eye = np.arange(G, dtype=np.int32)
    in_maps = []
    for c in range(NCORES):
        rows = slice(c * RPC, (c + 1) * RPC)
        a_c = _tile_tp(A[rows])
        b_rot = np.roll(B, -c * RPC, axis=0)
        lab_rot = np.roll(lab, -c * RPC)
        b_c = _tile_tp(b_rot)
        oha = np.concatenate(
            [
                np.ones((1, RPC), np.float32),
                (-BIG) * (lab[rows][None, :] == eye[:, None]).astype(np.float32),
            ]
        )
        ohb = np.concatenate(
            [
                np.full((1, N), 2.0 + BIG, np.float32),
                (lab_rot[None, :] == eye[:, None]).astype(np.float32),
            ]
        )
        in_maps.append(
            {
                "a": a_c,
                "b": b_c,
                "oha": np.ascontiguousarray(oha),
                "ohb": np.ascontiguousarray(ohb),
            }
        )

    global _last_in_maps, _last_nc
    _last_in_maps = in_maps
    nc = _get_nc()
    _last_nc = nc
    res = run_bass_kernel_spmd(nc, in_maps, list(range(NCORES)))
    total = 0.0
    for c in range(NCORES):
        lo = res.results[c]["losses"]  # [128, RT]; [p, r] = loss of row r*128+p
        total += float(lo.sum(dtype=np.float64))
    return np.float32(total / N)
